# revision 1
# baseline (speedup 1.0000x reference)
"""AlexNet-style CNN forward pass on 8 Trainium2 NeuronCores.

Strategy:
  - Convs data-parallel: batch 256 -> 32 per core, channels on partitions,
    conv = sum of shifted matmuls over kernel offsets (weights replicated).
  - conv1 (cin=3) uses host-packed im2col rows (3 dy-offsets x 11 dx x 3 ch
    + ones row for fused bias -> K=100) so the PE array is well utilized.
  - conv2 uses an x-shifted duplicate of its input (K=128 = 2 dx-offsets
    x 64 ch) to fill the contraction dim.
  - FC layers model-parallel: each core owns 512 rows of fc1/fc2 and 512
    K-columns of fc3; activations are AllGathered between layers, fc3
    partials AllReduced.  This cuts per-core FC weight DMA 8x.
  - Matmuls run as float32r (relaxed fp32, ~4x faster than strict fp32);
    everything else fp32.
"""

import ml_dtypes
import numpy as np

import concourse.bass as bass
import concourse.mybir as mybir
import concourse.tile as tile
from concourse import bacc
from concourse.bass_utils import run_bass_kernel_spmd

N_CORES = 8
B = 256
BC = B // N_CORES  # 32 images per core

F32 = mybir.dt.float32
F32R = mybir.dt.float32r
BF16 = mybir.dt.bfloat16
RELU = mybir.ActivationFunctionType.Relu
IDENT = mybir.ActivationFunctionType.Identity


def _r(ap):
    return ap.bitcast(F32R)


def _emit(nc, tc, t, yout):
    """Emit the whole network. t: dict name -> DRAM AP."""
    sync = nc.sync
    act = nc.scalar
    dve = nc.vector
    pool_e = nc.gpsimd

    psum = tc.alloc_tile_pool(name="psum", bufs=6, space="PSUM")
    scr = tc.alloc_tile_pool(name="scr", bufs=1, side="left")
    dram = tc.alloc_tile_pool(name="dram", bufs=1, space="DRAM")

    # ---------------- phase pools (queue alloc mode handles overlap) ----
    p_w12 = tc.alloc_tile_pool(name="p_w12", bufs=1, side="left")
    p_x2s = tc.alloc_tile_pool(name="p_x2s", bufs=1, side="left")
    p_x13 = tc.alloc_tile_pool(name="p_x13", bufs=2, side="left")

    # conv1+conv2 weights (host arrays already in SBUF layout)
    lw1 = p_w12.tile([100, 4 * 64], F32R)
    sync.dma_start(lw1[:], t["lw1"][:])
    lw2 = p_w12.tile([128, 7 * 4 * 192], F32R)
    sync.dma_start(lw2[:], t["lw2"][:])
    lb2 = p_w12.tile([128, 2], F32)
    sync.dma_start(lb2[:], t["lb2"][:])

    # conv2 input: [128, BC, 22, 23]; rows 0:64 ch c at x, rows 64:128 ch c at x+1
    X2s = p_x2s.tile([128, BC * 22 * 23], F32R)
    pool_e.memset(X2s[:].bitcast(F32), 0.0)

    def x2v(p0, p1, b0, nb, y0, ny, x0, nx):
        return X2s[p0:p1].rearrange("p (b y x) -> p b y x", b=BC, y=22, x=23)[
            :, b0:b0 + nb, y0:y0 + ny, x0:x0 + nx]

    # ---------------- conv1 + pool1 ----------------
    _sid = nc.enter_named_scope("L1_conv1", False)[0]
    for bg in range(4):  # groups of 8 images
        xt = p_x13.tile([100, 8 * 41 * 32], F32R, tag="x13")
        sync.dma_start(xt[:], t["x13"][bg])
        xtv = xt.rearrange("k (b y x) -> k b y x", b=8, y=41, x=32)
        for bl in range(8):
            b = bg * 8 + bl
            for h in range(2):  # vertical half of the 32x32 output
                ps = psum.tile([64, 512], F32, tag="ps")
                for pi, p in enumerate((0, 3, 6, 9)):
                    nc.tensor.matmul(
                        ps[:],
                        _r(lw1[:, pi * 64:(pi + 1) * 64]),
                        _r(xtv[:, bl, h * 16 + p:h * 16 + p + 16, :]),
                        start=(pi == 0), stop=(pi == 3),
                    )
                # evict+relu (bias came in via the ones-row), then 2x2 maxpool
                s1 = scr.tile([128, 512], F32, tag="ev", bufs=3)
                act.activation(s1[0:64, :], ps[:], RELU)
                s1v = s1[0:64, :].rearrange("m (y x) -> m y x", y=16, x=32)
                m1 = scr.tile([64, 128], F32, tag="m1", bufs=2)
                m2 = scr.tile([64, 128], F32, tag="m2", bufs=2)
                dve.tensor_max(m1[:], s1v[:, 0::2, 0::2], s1v[:, 0::2, 1::2])
                dve.tensor_max(m2[:], s1v[:, 1::2, 0::2], s1v[:, 1::2, 1::2])
                y0 = h * 8 + 3
                dve.tensor_max(
                    x2v(0, 64, b, 1, y0, 8, 3, 16)[:, 0],
                    m1.rearrange("m (y x) -> m y x", y=8, x=16),
                    m2.rearrange("m (y x) -> m y x", y=8, x=16))
        # duplicate this image-group into the x+1-shifted partition block
        # (engines cannot shift partitions; DMA can).  Flat-shifted copy:
        # one contiguous run per partition instead of 88B-strided rows; the
        # wrapped elements land in x=22 / come from x=0 pad, both dead.
        s0 = bg * 8 * 22 * 23
        ln = 8 * 22 * 23 - (1 if bg == 3 else 0)
        sync.dma_start(X2s[64:128, s0:s0 + ln], X2s[0:64, s0 + 1:s0 + 1 + ln])
    p_x13.release()
    nc.leave_named_scope("L1_conv1", _sid, False)

    # conv3 weights (prefetch during conv2) + conv3 input buffers
    p_w3 = tc.alloc_tile_pool(name="p_w3", bufs=1, side="right")
    p_x3 = tc.alloc_tile_pool(name="p_x3", bufs=1, side="right")
    lw3 = p_w3.tile([128, 15360], F32R)
    sync.dma_start(lw3[:], t["lw3"][:])
    lb3 = p_w3.tile([128, 3], F32)
    sync.dma_start(lb3[:], t["lb3"][:])
    X3a = p_x3.tile([128, BC * 12 * 12], F32R)
    # X3b rows 64:128 duplicate rows 0:64 so kc1 matmuls can run at
    # lhsT base_partition 64 (lw3 packs two kernel offsets per column block)
    X3b = p_x3.tile([128, BC * 12 * 12], F32R)
    pool_e.memset(X3a[:].bitcast(F32), 0.0)
    pool_e.memset(X3b[:].bitcast(F32), 0.0)

    def x3v(xab, p0, p1, b0, nb, y0, ny, x0, nx):
        return xab[p0:p1].rearrange("p (b y x) -> p b y x", b=BC, y=12, x=12)[
            :, b0:b0 + nb, y0:y0 + ny, x0:x0 + nx]

    # ---------------- conv2 + pool2 ----------------
    _sid = nc.enter_named_scope("L2_conv2", False)[0]
    lw2v = lw2.rearrange("k (a j m) -> k a j m", a=7, j=4, m=192)
    for nt in range(16):  # pairs of images
        for mc in range(2):
            M = 128 if mc == 0 else 64
            ps = psum.tile([M, 512], F32, tag="ps")
            first = True
            for dy in range(7):
                for j in range(4):
                    K = 128 if j < 3 else 64
                    xoff = 2 * j if j < 3 else 6
                    nc.tensor.matmul(
                        ps[:],
                        _r(lw2v[0:K, dy, j, mc * 128:mc * 128 + M]),
                        _r(x2v(0, K, nt * 2, 2, dy, 16, xoff, 16)),
                        start=first, stop=(dy == 6 and j == 3),
                    )
                    first = False
            s2 = scr.tile([128, 512], F32, tag="ev", bufs=3)
            act.activation(s2[:M], ps[:], RELU, bias=lb2[0:M, mc:mc + 1])
            s2v = s2.rearrange("m (b y x) -> m b y x", b=2, y=16, x=16)
            m1 = scr.tile([128, 128], F32, tag="m1", bufs=2)
            m2 = scr.tile([128, 128], F32, tag="m2", bufs=2)
            dve.tensor_max(m1[:M], s2v[:M, :, 0::2, 0::2], s2v[:M, :, 0::2, 1::2])
            dve.tensor_max(m2[:M], s2v[:M, :, 1::2, 0::2], s2v[:M, :, 1::2, 1::2])
            m1v = m1.rearrange("m (b y x) -> m b y x", b=2, y=8, x=8)
            m2v = m2.rearrange("m (b y x) -> m b y x", b=2, y=8, x=8)
            if mc == 0:
                dve.tensor_max(x3v(X3a, 0, 128, nt * 2, 2, 2, 8, 2, 8), m1v[:], m2v[:])
            else:
                dve.tensor_max(x3v(X3b, 0, 64, nt * 2, 2, 2, 8, 2, 8), m1v[:64], m2v[:64])
    # duplicate X3b into partitions 64:128 with a one-x shift (col x holds
    # value at x+1) so conv3's kc1 matmuls pair two dx offsets per K=128.
    # Flat copy: 64 big descriptors; wrap elements come from dead pad.
    sync.dma_start(X3b[64:128, 0:BC * 144 - 1], X3b[0:64, 1:BC * 144])
    nc.leave_named_scope("L2_conv2", _sid, False)
    p_x2s.release()
    p_w12.release()

    # conv4/5 weights (prefetch during conv3) + conv4 input buffers
    p_w45 = tc.alloc_tile_pool(name="p_w45", bufs=1, side="left")
    p_x4 = tc.alloc_tile_pool(name="p_x4", bufs=1, side="left")
    lw4 = p_w45.tile([128, 27 * 256], F32R)
    sync.dma_start(lw4[:], t["lw4"][:])
    lb4 = p_w45.tile([128, 2], F32)
    sync.dma_start(lb4[:], t["lb4"][:])
    lw5 = p_w45.tile([128, 18 * 256], F32R)
    sync.dma_start(lw5[:], t["lw5"][:])
    lb5 = p_w45.tile([128, 2], F32)
    sync.dma_start(lb5[:], t["lb5"][:])
    X4 = []
    for i in range(3):
        X4.append(p_x4.tile([128, BC * 10 * 10], F32R, name=f"X4_{i}"))
        pool_e.memset(X4[i][:].bitcast(F32), 0.0)

    def xv10(xab, p0, p1, b0, nb, y0, ny, x0, nx):
        return xab[p0:p1].rearrange("p (b y x) -> p b y x", b=BC, y=10, x=10)[
            :, b0:b0 + nb, y0:y0 + ny, x0:x0 + nx]

    _sid = nc.enter_named_scope("L3_conv3", False)[0]
    # ---------------- conv3 ----------------
    for nt in range(4):  # 8 images
        for mc in range(3):
            ps = psum.tile([128, 512], F32, tag="ps")
            first = True
            for dy in range(5):
                for dx in range(5):
                    blk = dy * 5 + dx
                    nc.tensor.matmul(
                        ps[:],
                        _r(lw3[0:128, blk * 384 + mc * 128:blk * 384 + mc * 128 + 128]),
                        _r(x3v(X3a, 0, 128, nt * 8, 8, dy, 8, dx, 8)),
                        start=first, stop=False,
                    )
                    first = False
                for j in range(3):  # kc1: dx pairs (0,1),(2,3),(4,)
                    K = 128 if j < 2 else 64
                    co = 9600 + (dy * 3 + j) * 384
                    nc.tensor.matmul(
                        ps[:],
                        _r(lw3[0:K, co + mc * 128:co + mc * 128 + 128]),
                        _r(x3v(X3b, 0, K, nt * 8, 8, dy, 8, 2 * j, 8)),
                        start=False, stop=(dy == 4 and j == 2),
                    )
            act.activation(
                xv10(X4[mc], 0, 128, nt * 8, 8, 1, 8, 1, 8),
                ps.rearrange("m (b y x) -> m b y x", b=8, y=8, x=8),
                RELU, bias=lb3[:, mc:mc + 1])
    nc.leave_named_scope("L3_conv3", _sid, False)
    p_x3.release()
    p_w3.release()

    # fc1 weights (prefetch during conv4) + conv5 input buffers
    p_fw1 = tc.alloc_tile_pool(name="p_fw1", bufs=1, side="right")
    p_x5 = tc.alloc_tile_pool(name="p_x5", bufs=1, side="right")
    fw1 = p_fw1.tile([128, 32 * 512], BF16)
    sync.dma_start(fw1[:], t["fw1s"][:])
    fb1 = p_fw1.tile([128, 4], F32)
    sync.dma_start(fb1[:], t["fb1s"][:])
    X5 = []
    for i in range(2):
        X5.append(p_x5.tile([128, BC * 10 * 10], F32R, name=f"X5_{i}"))
        pool_e.memset(X5[i][:].bitcast(F32), 0.0)

    _sid = nc.enter_named_scope("L4_conv4", False)[0]
    # ---------------- conv4 ----------------
    lw4v = lw4.rearrange("k (o m) -> k o m", o=27)
    for nt in range(4):
        for mc in range(2):
            ps = psum.tile([128, 512], F32, tag="ps")
            first = True
            for dy in range(3):
                for dx in range(3):
                    for kc in range(3):
                        o = (dy * 3 + dx) * 3 + kc
                        nc.tensor.matmul(
                            ps[:],
                            _r(lw4v[:, o, mc * 128:mc * 128 + 128]),
                            _r(xv10(X4[kc], 0, 128, nt * 8, 8, dy, 8, dx, 8)),
                            start=first, stop=(o == 26),
                        )
                        first = False
            act.activation(
                xv10(X5[mc], 0, 128, nt * 8, 8, 1, 8, 1, 8),
                ps.rearrange("m (b y x) -> m b y x", b=8, y=8, x=8),
                RELU, bias=lb4[:, mc:mc + 1])
    nc.leave_named_scope("L4_conv4", _sid, False)
    p_x4.release()

    # pool5 output
    p_p5 = tc.alloc_tile_pool(name="p_p5", bufs=1, side="left")
    P5 = [p_p5.tile([128, BC * 16], BF16, name=f"P5_{i}") for i in range(2)]

    # staging + gather buffers declared up front so each channel-half's
    # AllGather can launch inside conv5 and overlap the remaining compute
    cin5 = dram.tile([2, 128, BC * 16], BF16)
    g1 = [dram.tile([N_CORES, 128, BC * 16], BF16, addr_space="Shared",
                    name=f"g1cc{i}") for i in range(2)]

    _sid = nc.enter_named_scope("L5_conv5", False)[0]
    # ---------------- conv5 + pool5 ----------------
    lw5v = lw5.rearrange("k (o m) -> k o m", o=18)
    for mc in range(2):
        for nt in range(4):
            ps = psum.tile([128, 512], F32, tag="ps")
            first = True
            for dy in range(3):
                for dx in range(3):
                    for kc in range(2):
                        o = (dy * 3 + dx) * 2 + kc
                        nc.tensor.matmul(
                            ps[:],
                            _r(lw5v[:, o, mc * 128:mc * 128 + 128]),
                            _r(xv10(X5[kc], 0, 128, nt * 8, 8, dy, 8, dx, 8)),
                            start=first, stop=(o == 17),
                        )
                        first = False
            s5 = scr.tile([128, 512], F32, tag="ev", bufs=3)
            act.activation(s5[:], ps[:], RELU, bias=lb5[:, mc:mc + 1])
            s5v = s5.rearrange("m (b y x) -> m b y x", b=8, y=8, x=8)
            m1 = scr.tile([128, 128], F32, tag="m1", bufs=2)
            m2 = scr.tile([128, 128], F32, tag="m2", bufs=2)
            dve.tensor_max(m1[:], s5v[:, :, 0::2, 0::2], s5v[:, :, 0::2, 1::2])
            dve.tensor_max(m2[:], s5v[:, :, 1::2, 0::2], s5v[:, :, 1::2, 1::2])
            p5v = P5[mc].rearrange("p (b y x) -> p b y x", b=BC, y=4, x=4)
            dve.tensor_max(
                p5v[:, nt * 8:nt * 8 + 8, :, :],
                m1.rearrange("m (b y x) -> m b y x", b=8, y=4, x=4),
                m2.rearrange("m (b y x) -> m b y x", b=8, y=4, x=4))
        # this channel-half is complete: stage and gather it while the
        # other half (and the fc weight DMAs) still run
        sync.dma_start(cin5[mc], P5[mc][:])
        pool_e.collective_compute(
            "AllGather", mybir.AluOpType.bypass,
            replica_groups=[list(range(N_CORES))],
            ins=[cin5[mc].opt()], outs=[g1[mc].opt()])
    nc.leave_named_scope("L5_conv5", _sid, False)
    p_x5.release()
    p_p5.release()
    p_w45.release()

    # fc2/fc3 weights (DMA overlaps the gather + fc1)
    p_fw2 = tc.alloc_tile_pool(name="p_fw2", bufs=1, side="left")
    fw2 = p_fw2.tile([128, 32 * 512], BF16)
    sync.dma_start(fw2[:], t["fw2s"][:])
    fb2 = p_fw2.tile([128, 4], F32)
    sync.dma_start(fb2[:], t["fb2s"][:])
    fw3 = p_fw2.tile([128, 4 * 100], BF16)
    sync.dma_start(fw3[:], t["fw3s"][:])
    fb3 = p_fw2.tile([100, 1], F32)
    sync.dma_start(fb3[:], t["fb3s"][:])

    _sid = nc.enter_named_scope("G1_gather", False)[0]
    # ---------------- load gathered pool5 -> fc input ----------------
    p_h1 = tc.alloc_tile_pool(name="p_h1", bufs=1, side="right")
    H1 = [p_h1.tile([128, N_CORES * BC * 16], BF16, name=f"H1_{i}") for i in range(2)]
    for cc in range(2):
        sync.dma_start(
            H1[cc].rearrange("c (r f) -> c r f", r=N_CORES),
            g1[cc].rearrange("r c f -> c r f"))

    nc.leave_named_scope("G1_gather", _sid, False)
    _sid = nc.enter_named_scope("F1_fc1", False)[0]
    # ---------------- fc1 (model-parallel over 512 outputs) ----------------
    p_f1 = tc.alloc_tile_pool(name="p_f1", bufs=1, side="left")
    F1 = p_f1.tile([128, 4 * B], BF16)
    fw1v = fw1.rearrange("k (y c m) -> k y c m", y=16, c=2, m=512)
    for mc in range(4):
        ps = psum.tile([128, B], F32, tag="ps")
        first = True
        for yx in range(16):
            for cc in range(2):
                rhs = H1[cc].rearrange("c (r b y) -> c y r b", r=N_CORES, b=BC, y=16)
                nc.tensor.matmul(
                    ps[:],
                    fw1v[:, yx, cc, mc * 128:mc * 128 + 128],
                    rhs[:, yx],
                    start=first, stop=(yx == 15 and cc == 1))
                first = False
        act.activation(F1[:, mc * B:(mc + 1) * B], ps[:], RELU, bias=fb1[:, mc:mc + 1])
    p_h1.release()
    p_fw1.release()

    nc.leave_named_scope("F1_fc1", _sid, False)
    _sid = nc.enter_named_scope("G2_gather", False)[0]
    # ---------------- AllGather fc1 ----------------
    cin6 = dram.tile([128, 4 * B], BF16)
    sync.dma_start(cin6[:], F1[:])
    g2 = dram.tile([N_CORES, 128, 4 * B], BF16, addr_space="Shared")
    pool_e.collective_compute(
        "AllGather", mybir.AluOpType.bypass,
        replica_groups=[list(range(N_CORES))],
        ins=[cin6.opt()], outs=[g2.opt()])
    p_f1.release()

    p_h2 = tc.alloc_tile_pool(name="p_h2", bufs=1, side="right")
    H2 = p_h2.tile([128, N_CORES * 4 * B], BF16)
    sync.dma_start(
        H2.rearrange("c (r f) -> c r f", r=N_CORES),
        g2.rearrange("r c f -> c r f"))

    nc.leave_named_scope("G2_gather", _sid, False)
    _sid = nc.enter_named_scope("F2_fc2", False)[0]
    # ---------------- fc2 ----------------
    p_f2 = tc.alloc_tile_pool(name="p_f2", bufs=1, side="left")
    F2 = p_f2.tile([128, 4 * B], BF16)
    fw2v = fw2.rearrange("k (a m) -> k a m", a=32)
    for mc in range(4):
        ps = psum.tile([128, B], F32, tag="ps")
        for kc in range(32):
            nc.tensor.matmul(
                ps[:], fw2v[:, kc, mc * 128:mc * 128 + 128],
                H2[:, kc * B:(kc + 1) * B],
                start=(kc == 0), stop=(kc == 31))
        act.activation(F2[:, mc * B:(mc + 1) * B], ps[:], RELU, bias=fb2[:, mc:mc + 1])
    p_h2.release()

    nc.leave_named_scope("F2_fc2", _sid, False)
    _sid = nc.enter_named_scope("F3_fc3", False)[0]
    # ---------------- fc3 (partial over this core's 512 K) + AllReduce ----
    fw3v = fw3.rearrange("k (a m) -> k a m", a=4)
    ps = psum.tile([128, B], F32, tag="ps")
    for kc in range(4):
        nc.tensor.matmul(
            ps[0:100, :], fw3v[:, kc, :], F2[:, kc * B:(kc + 1) * B],
            start=(kc == 0), stop=(kc == 3))
    s3 = scr.tile([128, 512], F32, tag="ev", bufs=3)
    act.activation(s3[0:100, 0:B], ps[0:100, :], IDENT, bias=fb3[:])  # + fb3/8
    cin7 = dram.tile([100, B], F32)
    sync.dma_start(cin7[:], s3[0:100, 0:B])
    g3 = dram.tile([100, B], F32)
    pool_e.collective_compute(
        "AllReduce", mybir.AluOpType.add,
        replica_groups=[list(range(N_CORES))],
        ins=[cin7.opt()], outs=[g3.opt()])
    sync.dma_start(yout[:], g3[:])
    nc.leave_named_scope("F3_fc3", _sid, False)
    p_f2.release()
    p_fw2.release()

    scr.release()
    dram.release()
    psum.release()


# ---------------------------------------------------------------------------
# host-side input prep (numpy; all weight arrays already in SBUF layout)
# ---------------------------------------------------------------------------

def _prep_shared(w1, b1, w2, b2, w3, b3, w4, b4, w5, b5):
    f = np.float32
    # conv1: rows r = dyo*33 + dx*3 + c, row 99 = bias(ones); 4 passes dy=3p+dyo
    lw1 = np.zeros((100, 4 * 64), f)
    for p in range(4):
        for dyo in range(3):
            dy = 3 * p + dyo
            if dy > 10:
                continue
            for dx in range(11):
                for c in range(3):
                    lw1[dyo * 33 + dx * 3 + c, p * 64:(p + 1) * 64] = w1[:, c, dy, dx]
    lw1[99, 0:64] = b1
    # conv2: [128, (dy,j,m)]: j<3 -> rows s*64+c = ch c at dx=2j+s; j=3 -> dx=6
    lw2 = np.zeros((128, 7 * 4 * 192), f)
    for dy in range(7):
        for j in range(3):
            for s in range(2):
                lw2[s * 64:(s + 1) * 64, (dy * 4 + j) * 192:(dy * 4 + j + 1) * 192] = \
                    w2[:, :, dy, 2 * j + s].T
        lw2[0:64, (dy * 4 + 3) * 192:(dy * 4 + 4) * 192] = w2[:, :, dy, 6].T
    lb2 = np.zeros((128, 2), f)
    lb2[:, 0] = b2[0:128]
    lb2[0:64, 1] = b2[128:192]
    # conv3: [128, 15360]: cols blk*384+m (kc0, K=128); kc1 packs dx pairs
    # (0,1),(2,3),(4,) per dy at 9600+(dy*3+j)*384 — rows 0:64 = dx=2j,
    # rows 64:128 = dx=2j+1 (X3b partitions 64:128 hold x+1-shifted acts)
    lw3 = np.zeros((128, 15360), f)
    for dy in range(5):
        for dx in range(5):
            blk = dy * 5 + dx
            lw3[:, blk * 384:(blk + 1) * 384] = w3[:, 0:128, dy, dx].T
        for j in range(3):
            co = 9600 + (dy * 3 + j) * 384
            lw3[0:64, co:co + 384] = w3[:, 128:192, dy, 2 * j].T
            if j < 2:
                lw3[64:128, co:co + 384] = w3[:, 128:192, dy, 2 * j + 1].T
    lb3 = np.zeros((128, 3), f)
    lb3[:, 0] = b3[0:128]; lb3[:, 1] = b3[128:256]; lb3[:, 2] = b3[256:384]
    # conv4 / conv5: [128, (o, m)] with o = (dy*3+dx)*nkc + kc
    lw4 = np.zeros((128, 27 * 256), f)
    for dy in range(3):
        for dx in range(3):
            for kc in range(3):
                o = (dy * 3 + dx) * 3 + kc
                lw4[:, o * 256:(o + 1) * 256] = w4[:, kc * 128:(kc + 1) * 128, dy, dx].T
    lb4 = np.stack([b4[0:128], b4[128:256]], axis=1).astype(f)
    lw5 = np.zeros((128, 18 * 256), f)
    for dy in range(3):
        for dx in range(3):
            for kc in range(2):
                o = (dy * 3 + dx) * 2 + kc
                lw5[:, o * 256:(o + 1) * 256] = w5[:, kc * 128:(kc + 1) * 128, dy, dx].T
    lb5 = np.stack([b5[0:128], b5[128:256]], axis=1).astype(f)
    return dict(lw1=lw1, lw2=lw2, lb2=lb2, lw3=lw3, lb3=lb3,
                lw4=lw4, lb4=lb4, lw5=lw5, lb5=lb5)


def _prep_x13(x):
    """x [B,3,32,32] -> per-core [4, 100, 8*41*32] im2col-packed."""
    f = np.float32
    xpad = np.zeros((B, 3, 44, 42), f)
    xpad[:, :, 5:37, 5:37] = x
    X = np.zeros((100, B, 41, 32), f)
    for dyo in range(3):
        for dx in range(11):
            for c in range(3):
                X[dyo * 33 + dx * 3 + c] = xpad[:, c, dyo:dyo + 41, dx:dx + 32]
    X[99] = 1.0
    out = []
    for r in range(N_CORES):
        pc = X[:, r * BC:(r + 1) * BC]  # [100, 32, 41, 32]
        pc = pc.reshape(100, 4, 8 * 41 * 32).transpose(1, 0, 2)
        out.append(np.ascontiguousarray(pc))
    return out


def _prep_fc(fw1, fb1, fw2, fb2, fw3, fb3):
    f = np.float32
    outs = []
    for r in range(N_CORES):
        sl = slice(512 * r, 512 * (r + 1))
        # fw1s [128, (yx, cc, m)]: fw1[512r+m, (cc*128+k)*16+yx]
        fw1s = fw1[sl].reshape(512, 2, 128, 16).transpose(2, 3, 1, 0).reshape(128, -1)
        fb1s = fb1[sl].reshape(4, 128).T
        # fw2s [128, (kc, m)]: fw2[512r+m, kc*128+k]
        fw2s = fw2[sl].reshape(512, 32, 128).transpose(2, 1, 0).reshape(128, -1)
        fb2s = fb2[sl].reshape(4, 128).T
        # fw3s [128, (kc, m)]: fw3[m, 512r + kc*128 + k]
        fw3s = fw3[:, sl].reshape(100, 4, 128).transpose(2, 1, 0).reshape(128, -1)
        fb3s = (fb3 / N_CORES).reshape(100, 1)
        bf = ml_dtypes.bfloat16
        outs.append(dict(
            fw1s=np.ascontiguousarray(fw1s.astype(bf)),
            fb1s=np.ascontiguousarray(fb1s.astype(f)),
            fw2s=np.ascontiguousarray(fw2s.astype(bf)),
            fb2s=np.ascontiguousarray(fb2s.astype(f)),
            fw3s=np.ascontiguousarray(fw3s.astype(bf)),
            fb3s=np.ascontiguousarray(fb3s.astype(f)),
        ))
    return outs


_CACHE = {}

_SHAPES = dict(
    x13=(4, 100, 8 * 41 * 32), lw1=(100, 4 * 64),
    lw2=(128, 7 * 4 * 192), lb2=(128, 2),
    lw3=(128, 15360), lb3=(128, 3),
    lw4=(128, 27 * 256), lb4=(128, 2),
    lw5=(128, 18 * 256), lb5=(128, 2),
    fw1s=(128, 32 * 512), fb1s=(128, 4),
    fw2s=(128, 32 * 512), fb2s=(128, 4),
    fw3s=(128, 4 * 100), fb3s=(100, 1),
)


def _build():
    if "nc" in _CACHE:
        return _CACHE["nc"]
    nc = bacc.Bacc("TRN2", target_bir_lowering=False, debug=False,
                   num_devices=N_CORES)
    _F32R_INPUTS = {"x13", "lw1", "lw2", "lw3", "lw4", "lw5"}
    _BF16_INPUTS = {"fw1s", "fw2s", "fw3s"}

    def _dt(name):
        if name in _F32R_INPUTS:
            return F32R
        if name in _BF16_INPUTS:
            return BF16
        return F32
    t = {name: nc.dram_tensor(
            name, list(shape), _dt(name), kind="ExternalInput").ap()
         for name, shape in _SHAPES.items()}
    yout = nc.dram_tensor("yout", [100, B], F32, kind="ExternalOutput").ap()
    with tile.TileContext(nc) as tc:
        _emit(nc, tc, t, yout)
    nc.compile()
    _CACHE["nc"] = nc
    return nc


def kernel(x, w1, b1, w2, b2, w3, b3, w4, b4, w5, b5,
           fw1, fb1, fw2, fb2, fw3, fb3):
    args = [np.asarray(a, np.float32) for a in
            (x, w1, b1, w2, b2, w3, b3, w4, b4, w5, b5, fw1, fb1, fw2, fb2, fw3, fb3)]
    (x, w1, b1, w2, b2, w3, b3, w4, b4, w5, b5,
     fw1, fb1, fw2, fb2, fw3, fb3) = args
    nc = _build()
    shared = _prep_shared(w1, b1, w2, b2, w3, b3, w4, b4, w5, b5)
    x13s = _prep_x13(x)
    fcs = _prep_fc(fw1, fb1, fw2, fb2, fw3, fb3)
    in_maps = [{**shared, "x13": x13s[r], **fcs[r]} for r in range(N_CORES)]
    res = run_bass_kernel_spmd(nc, in_maps, list(range(N_CORES)))
    y = res.results[0]["yout"]  # [100, 256]
    return np.ascontiguousarray(y.T)



# revision 17
# speedup vs baseline: 1.3493x; 1.3493x over previous
"""AlexNet-style CNN forward pass on 8 Trainium2 NeuronCores.

Strategy (v1):
  - Convs data-parallel: batch 256 -> 32 per core, channels on partitions,
    conv = sum of shifted matmuls over kernel offsets (weights replicated).
  - All conv weights/activations in bf16 (PSUM accumulates fp32); halves
    DMA + SBUF traffic and enables FWL weight loads.
  - Activations laid out batch-innermost ([ch, y, x, b]) so every matmul
    rhs is runs of 32 contiguous elements (the fp32 baseline's 8-elem
    strided runs slowed the PE rhs stream ~1.3x).
  - conv1 (cin=3) uses host-packed im2col rows (3 dy-offsets x 11 dx x 3 ch
    + ones row for fused bias -> K=100), 8 input chunks DMA'd on the sync
    queue (weights go on the scalar queue) so the first matmul starts ~8us
    in instead of 53us.
  - conv2 uses an x-shifted duplicate of its input (K=128 = 2 dx-offsets
    x 64 ch); conv3 likewise packs two dx offsets via a shifted copy.
  - FC layers model-parallel: each core owns 512 rows of fc1/fc2 and 512
    K-columns of fc3.  pool5 is AllGathered in 2 chunks that overlap fc1's
    K-accumulation into persistent PSUM; fc1 output AllGathered, fc3
    partials AllReduced.
"""

import os

import ml_dtypes
import numpy as np

import concourse.bass as bass
import concourse.mybir as mybir
import concourse.tile as tile
from concourse import bacc
from concourse.bass_utils import run_bass_kernel_spmd

N_CORES = 8
B = 256
BC = B // N_CORES  # 32 images per core

F32 = mybir.dt.float32
BF16 = mybir.dt.bfloat16
RELU = mybir.ActivationFunctionType.Relu
IDENT = mybir.ActivationFunctionType.Identity

DEBUG_DUMPS = os.environ.get("BASSDBG") == "1"

_DBG_SHAPES = dict(
    dX2s=(128, 22 * 23 * 32), dX3a=(128, 12 * 12 * 32), dX3b=(128, 12 * 12 * 32),
    dX4_0=(128, 3200), dX4_1=(128, 3200), dX4_2=(128, 3200),
    dX5_0=(128, 3200), dX5_1=(128, 3200),
    dP5_0=(128, 512), dP5_1=(128, 512),
    dF1=(128, 1024), dF2=(128, 1024),
)


def _emit(nc, tc, t, yout):
    """Emit the whole network. t: dict name -> DRAM AP."""
    sync = nc.sync
    act = nc.scalar
    dve = nc.vector
    pool_e = nc.gpsimd

    psum = tc.alloc_tile_pool(name="psum", bufs=6, space="PSUM")
    scr = tc.alloc_tile_pool(name="scr", bufs=1, side="left")
    dram = tc.alloc_tile_pool(name="dram", bufs=1, space="DRAM")

    # ---------------- phase pools ----------------
    p_w12 = tc.alloc_tile_pool(name="p_w12", bufs=1, side="left")
    p_x2s = tc.alloc_tile_pool(name="p_x2s", bufs=1, side="left")
    p_x13 = tc.alloc_tile_pool(name="p_x13", bufs=3, side="left")

    # first x13 chunk DMA goes FIRST on the sync queue so conv1 can start
    xt0 = p_x13.tile([100, 4 * 41 * 32], BF16, tag="x13")
    sync.dma_start(xt0[:], t["x13"][0])

    # conv1+conv2 weights on the scalar queue (parallel with x13 stream)
    lw1 = p_w12.tile([100, 4 * 64], BF16)
    act.dma_start(lw1[:], t["lw1"][:])
    lw2 = p_w12.tile([128, 7 * 4 * 192], BF16)
    act.dma_start(lw2[:], t["lw2"][:])
    lb2 = p_w12.tile([128, 2], F32)
    act.dma_start(lb2[:], t["lb2"][:])

    # conv2 input: [128, 22y, 23x, 32b]; rows 0:64 ch c at x, rows 64:128
    # ch c at x+1 (b innermost so matmul rhs reads runs of 32 elems)
    X2s = p_x2s.tile([128, 22 * 23 * 32], BF16)
    pool_e.memset(X2s[:].bitcast(F32), 0.0)
    X2sv = X2s.rearrange("p (y x b) -> p y x b", y=22, x=23, b=32)

    # ---------------- conv1 + pool1 ----------------
    _sid = nc.enter_named_scope("L1_conv1", False)[0]
    for ch in range(8):  # chunks of 4 images
        if ch == 0:
            xt = xt0
        else:
            xt = p_x13.tile([100, 4 * 41 * 32], BF16, tag="x13")
            sync.dma_start(xt[:], t["x13"][ch])
        xtv = xt.rearrange("k (b y x) -> k b y x", b=4, y=41, x=32)
        for bl in range(4):
            b = ch * 4 + bl
            for h in range(2):  # vertical half of the 32x32 output
                ps = psum.tile([64, 512], F32, tag="ps")
                for pi, p in enumerate((0, 3, 6, 9)):
                    nc.tensor.matmul(
                        ps[:],
                        lw1[:, pi * 64:(pi + 1) * 64],
                        xtv[:, bl, h * 16 + p:h * 16 + p + 16, :],
                        start=(pi == 0), stop=(pi == 3),
                    )
                # evict+relu (bias came in via the ones-row), then 2x2 maxpool
                s1 = scr.tile([64, 512], BF16, tag="ev1", bufs=3)
                act.activation(s1[:], ps[:], RELU)
                s1v = s1.rearrange("m (y x) -> m y x", y=16, x=32)
                m1 = scr.tile([64, 128], BF16, tag="m1", bufs=2)
                m2 = scr.tile([64, 128], BF16, tag="m2", bufs=2)
                dve.tensor_max(m1[:], s1v[:, 0::2, 0::2], s1v[:, 0::2, 1::2])
                dve.tensor_max(m2[:], s1v[:, 1::2, 0::2], s1v[:, 1::2, 1::2])
                y0 = h * 8 + 3
                dve.tensor_max(
                    X2sv[0:64, y0:y0 + 8, 3:19, b],
                    m1.rearrange("m (y x) -> m y x", y=8, x=16),
                    m2.rearrange("m (y x) -> m y x", y=8, x=16))
    # duplicate into the x+1-shifted partition block (b-inner: shift by 32
    # elems).  Flat-shifted copy in 2 y-chunks so conv2's first rows can
    # start while the second chunk is in flight; wrapped elements land in
    # dead pad columns.
    HALF = 11 * 23 * 32
    TOT = 22 * 23 * 32
    sync.dma_start(X2s[64:128, 0:HALF], X2s[0:64, 32:HALF + 32])
    sync.dma_start(X2s[64:128, HALF:TOT - 32], X2s[0:64, HALF + 32:TOT])
    p_x13.release()
    nc.leave_named_scope("L1_conv1", _sid, False)

    # conv3 weights (prefetch during conv2) + conv3 input buffers
    p_w3 = tc.alloc_tile_pool(name="p_w3", bufs=1, side="right")
    p_x3 = tc.alloc_tile_pool(name="p_x3", bufs=1, side="right")
    lw3 = p_w3.tile([128, 15360], BF16)
    act.dma_start(lw3[:], t["lw3"][:])
    lb3 = p_w3.tile([128, 3], F32)
    act.dma_start(lb3[:], t["lb3"][:])
    # X3a: [128ch, 12y, 12x, 32b] (pad 2); X3b rows 0:64 = ch 128:192,
    # rows 64:128 its x+1-shifted copy (conv3 kc1 packs two dx per K=128)
    X3a = p_x3.tile([128, 12 * 12 * 32], BF16)
    X3b = p_x3.tile([128, 12 * 12 * 32], BF16)
    pool_e.memset(X3a[:].bitcast(F32), 0.0)
    pool_e.memset(X3b[:].bitcast(F32), 0.0)
    X3av = X3a.rearrange("p (y x b) -> p y x b", y=12, x=12, b=32)
    X3bv = X3b.rearrange("p (y x b) -> p y x b", y=12, x=12, b=32)

    # ---------------- conv2 + pool2 ----------------
    _sid = nc.enter_named_scope("L2_conv2", False)[0]
    lw2v = lw2.rearrange("k (a j m) -> k a j m", a=7, j=4, m=192)
    s2keep = {}
    for y in range(16):  # output rows; psum free dim = 16x * 32b
        for mc in range(2):
            M = 128 if mc == 0 else 64
            ps = psum.tile([M, 512], F32, tag="ps")
            first = True
            for dy in range(7):
                for j in range(4):
                    K = 128 if j < 3 else 64
                    xoff = 2 * j if j < 3 else 6
                    nc.tensor.matmul(
                        ps[:],
                        lw2v[0:K, dy, j, mc * 128:mc * 128 + M],
                        X2sv[0:K, y + dy, xoff:xoff + 16, :],
                        start=first, stop=(dy == 6 and j == 3),
                    )
                    first = False
            s2 = scr.tile([128, 512], BF16, tag="ev", bufs=4)
            act.activation(s2[:M], ps[:], RELU, bias=lb2[0:M, mc:mc + 1])
            if y % 2 == 0:
                s2keep[mc] = s2
            else:
                # 2x2 maxpool: vertical (row pair), then horizontal
                pm = scr.tile([128, 512], BF16, tag="pm", bufs=2)
                dve.tensor_max(pm[:M], s2keep[mc][:M], s2[:M])
                pmv = pm.rearrange("m (x b) -> m x b", x=16, b=32)
                tdst = (X3av[0:128, y // 2 + 2, 2:10, :] if mc == 0
                        else X3bv[0:64, y // 2 + 2, 2:10, :])
                dve.tensor_max(tdst, pmv[:M, 0::2, :], pmv[:M, 1::2, :])
    # x+1-shifted duplicate for conv3's dx pairing (flat +32 shift)
    sync.dma_start(X3b[64:128, 0:12 * 12 * 32 - 32], X3b[0:64, 32:12 * 12 * 32])
    if DEBUG_DUMPS:
        sync.dma_start(t["dX2s"][:], X2s[:])
        sync.dma_start(t["dX3a"][:], X3a[:])
        sync.dma_start(t["dX3b"][:], X3b[:])
    nc.leave_named_scope("L2_conv2", _sid, False)
    p_x2s.release()
    p_w12.release()

    # conv4/5 weights (prefetch during conv3) + conv4 input buffers
    p_w45 = tc.alloc_tile_pool(name="p_w45", bufs=1, side="left")
    p_x4 = tc.alloc_tile_pool(name="p_x4", bufs=1, side="left")
    lw4 = p_w45.tile([128, 27 * 256], BF16)
    act.dma_start(lw4[:], t["lw4"][:])
    lb4 = p_w45.tile([128, 2], F32)
    act.dma_start(lb4[:], t["lb4"][:])
    lw5 = p_w45.tile([128, 18 * 256], BF16)
    act.dma_start(lw5[:], t["lw5"][:])
    lb5 = p_w45.tile([128, 2], F32)
    act.dma_start(lb5[:], t["lb5"][:])
    X4 = []
    X4v = []
    for i in range(3):
        X4.append(p_x4.tile([128, 10 * 10 * 32], BF16, name=f"X4_{i}"))
        pool_e.memset(X4[i][:].bitcast(F32), 0.0)
        X4v.append(X4[i].rearrange("p (y x b) -> p y x b", y=10, x=10, b=32))

    _sid = nc.enter_named_scope("L3_conv3", False)[0]
    # ---------------- conv3 ----------------
    for tt in range(4):  # output row pairs; psum free = 2y * 8x * 32b
        for mc in range(3):
            ps = psum.tile([128, 512], F32, tag="ps")
            first = True
            for dy in range(5):
                for dx in range(5):
                    blk = dy * 5 + dx
                    nc.tensor.matmul(
                        ps[:],
                        lw3[0:128, blk * 384 + mc * 128:blk * 384 + mc * 128 + 128],
                        X3av[0:128, 2 * tt + dy:2 * tt + dy + 2, dx:dx + 8, :],
                        start=first, stop=False,
                    )
                    first = False
                for j in range(3):  # kc1: dx pairs (0,1),(2,3),(4,)
                    K = 128 if j < 2 else 64
                    co = 9600 + (dy * 3 + j) * 384
                    nc.tensor.matmul(
                        ps[:],
                        lw3[0:K, co + mc * 128:co + mc * 128 + 128],
                        X3bv[0:K, 2 * tt + dy:2 * tt + dy + 2, 2 * j:2 * j + 8, :],
                        start=False, stop=(dy == 4 and j == 2),
                    )
            act.activation(
                X4v[mc][:, 2 * tt + 1:2 * tt + 3, 1:9, :],
                ps.rearrange("m (y x b) -> m y x b", y=2, x=8, b=32),
                RELU, bias=lb3[:, mc:mc + 1])
    if DEBUG_DUMPS:
        for i in range(3):
            sync.dma_start(t[f"dX4_{i}"][:], X4[i][:])
    nc.leave_named_scope("L3_conv3", _sid, False)
    p_x3.release()
    p_w3.release()

    # fc1 weights (prefetch during conv4) + conv5 input buffers
    p_fw1 = tc.alloc_tile_pool(name="p_fw1", bufs=1, side="right")
    p_x5 = tc.alloc_tile_pool(name="p_x5", bufs=1, side="right")
    fw1 = p_fw1.tile([128, 32 * 512], BF16)
    act.dma_start(fw1[:], t["fw1s"][:])
    fb1 = p_fw1.tile([128, 4], F32)
    act.dma_start(fb1[:], t["fb1s"][:])
    X5 = []
    X5v = []
    for i in range(2):
        X5.append(p_x5.tile([128, 10 * 10 * 32], BF16, name=f"X5_{i}"))
        pool_e.memset(X5[i][:].bitcast(F32), 0.0)
        X5v.append(X5[i].rearrange("p (y x b) -> p y x b", y=10, x=10, b=32))

    _sid = nc.enter_named_scope("L4_conv4", False)[0]
    # ---------------- conv4 ----------------
    lw4v = lw4.rearrange("k (o m) -> k o m", o=27)
    for tt in range(4):
        for mc in range(2):
            ps = psum.tile([128, 512], F32, tag="ps")
            first = True
            for dy in range(3):
                for dx in range(3):
                    for kc in range(3):
                        o = (dy * 3 + dx) * 3 + kc
                        nc.tensor.matmul(
                            ps[:],
                            lw4v[:, o, mc * 128:mc * 128 + 128],
                            X4v[kc][:, 2 * tt + dy:2 * tt + dy + 2, dx:dx + 8, :],
                            start=first, stop=(o == 26),
                        )
                        first = False
            act.activation(
                X5v[mc][:, 2 * tt + 1:2 * tt + 3, 1:9, :],
                ps.rearrange("m (y x b) -> m y x b", y=2, x=8, b=32),
                RELU, bias=lb4[:, mc:mc + 1])
    if DEBUG_DUMPS:
        for i in range(2):
            sync.dma_start(t[f"dX5_{i}"][:], X5[i][:])
    nc.leave_named_scope("L4_conv4", _sid, False)
    p_x4.release()

    # fc2/fc3 weights (DMA overlaps conv5 + the gathers + fc1)
    p_fw2 = tc.alloc_tile_pool(name="p_fw2", bufs=1, side="left")
    fw2 = p_fw2.tile([128, 32 * 512], BF16)
    act.dma_start(fw2[:], t["fw2s"][:])
    fb2 = p_fw2.tile([128, 4], F32)
    act.dma_start(fb2[:], t["fb2s"][:])
    fw3 = p_fw2.tile([128, 4 * 100], BF16)
    act.dma_start(fw3[:], t["fw3s"][:])
    fb3 = p_fw2.tile([100, 1], F32)
    act.dma_start(fb3[:], t["fb3s"][:])

    # pool5 output: [128ch, 4t, 4x, 32b] per channel half
    p_p5 = tc.alloc_tile_pool(name="p_p5", bufs=1, side="left")
    P5 = [p_p5.tile([128, BC * 16], BF16, name=f"P5_{i}") for i in range(2)]

    # fc1 persistent psum: 4 mc-quarter outputs of [128, 256] in 2 banks
    p_psF = tc.alloc_tile_pool(name="psumF", bufs=1, space="PSUM")
    psF = p_psF.tile([128, 1024], F32)

    # staging + gather buffers for the 2-chunk pool5 AllGather
    cin5 = [dram.tile([2, 2, 128, 128], BF16, name=f"cin5_{i}") for i in range(2)]
    g1 = [dram.tile([N_CORES, 2, 2, 128, 128], BF16, addr_space="Shared",
                    name=f"g1cc{i}") for i in range(2)]

    _sid = nc.enter_named_scope("L5_conv5", False)[0]
    # ---------------- conv5 + pool5 (+ chunked gather launch) ----------------
    lw5v = lw5.rearrange("k (o m) -> k o m", o=18)
    for tt in range(4):
        for mc in range(2):
            ps = psum.tile([128, 512], F32, tag="ps")
            first = True
            for dy in range(3):
                for dx in range(3):
                    for kc in range(2):
                        o = (dy * 3 + dx) * 2 + kc
                        nc.tensor.matmul(
                            ps[:],
                            lw5v[:, o, mc * 128:mc * 128 + 128],
                            X5v[kc][:, 2 * tt + dy:2 * tt + dy + 2, dx:dx + 8, :],
                            start=first, stop=(o == 17),
                        )
                        first = False
            s5 = scr.tile([128, 512], BF16, tag="ev", bufs=4)
            act.activation(s5[:], ps[:], RELU, bias=lb5[:, mc:mc + 1])
            s5v = s5.rearrange("m (y x b) -> m y x b", y=2, x=8, b=32)
            pm = scr.tile([128, 256], BF16, tag="pm5", bufs=2)
            dve.tensor_max(pm[:], s5v[:, 0, :, :], s5v[:, 1, :, :])
            pmv = pm.rearrange("m (x b) -> m x b", x=8, b=32)
            p5v = P5[mc].rearrange("p (t x b) -> p t x b", t=4, x=4, b=32)
            dve.tensor_max(p5v[:, tt, :, :], pmv[:, 0::2, :], pmv[:, 1::2, :])
        if tt % 2 == 1:
            # stage this pair of row-chunks and launch its AllGather; the
            # first chunk's gather overlaps the second half of conv5
            cc = tt // 2
            for si in range(2):
                for mc in range(2):
                    sync.dma_start(
                        cin5[cc][si, mc],
                        P5[mc][:, (2 * cc + si) * 128:(2 * cc + si + 1) * 128])
            pool_e.collective_compute(
                "AllGather", mybir.AluOpType.bypass,
                replica_groups=[list(range(N_CORES))],
                ins=[cin5[cc].opt()], outs=[g1[cc].opt()])
    if DEBUG_DUMPS:
        for i in range(2):
            sync.dma_start(t[f"dP5_{i}"][:], P5[i][:])
    nc.leave_named_scope("L5_conv5", _sid, False)
    p_x5.release()

    _sid = nc.enter_named_scope("F1_fc1", False)[0]
    # ---------------- fc1: accumulate gathered chunks into psF ----------------
    p_h1 = tc.alloc_tile_pool(name="p_h1", bufs=2, side="right")
    p_f1 = tc.alloc_tile_pool(name="p_f1", bufs=1, side="left")
    F1 = p_f1.tile([128, 4 * B], BF16)
    fw1v = fw1.rearrange("k (y c m) -> k y c m", y=16, c=2, m=512)
    for cc in range(2):  # gathered chunk (row pair of pool5)
        H1 = p_h1.tile([128, 2 * 2 * 8 * 128], BF16, tag="h1")
        H1v = H1.rearrange("c (t m r f) -> c t m r f", t=2, m=2, r=8, f=128)
        for si in range(2):
            for mc in range(2):
                sync.dma_start(
                    H1v[:, si, mc],
                    g1[cc][:, si, mc].rearrange("r c f -> c r f"))
        for si in range(2):
            trow = 2 * cc + si
            for x in range(4):
                for half in range(2):
                    yx = trow * 4 + x
                    for mq in range(4):
                        # start=True clears has_written for the WHOLE bank;
                        # mq pairs (0,1) and (2,3) share a bank, so only the
                        # bank's first matmul may set it — the partner region
                        # inits via overwrite-where-unset semantics.
                        nc.tensor.matmul(
                            psF[:, mq * 256:(mq + 1) * 256],
                            fw1v[:, yx, half, mq * 128:mq * 128 + 128],
                            H1v[:, si, half, :, x * 32:(x + 1) * 32],
                            start=(cc == 0 and si == 0 and x == 0
                                   and half == 0 and mq % 2 == 0),
                            stop=(cc == 1 and si == 1 and x == 3 and half == 1),
                            skip_group_check=True,
                        )
    for mq in range(4):
        act.activation(F1[:, mq * B:(mq + 1) * B],
                       psF[:, mq * 256:(mq + 1) * 256],
                       RELU, bias=fb1[:, mq:mq + 1])
    if DEBUG_DUMPS:
        sync.dma_start(t["dF1"][:], F1[:])
    p_h1.release()
    p_fw1.release()
    p_psF.release()

    nc.leave_named_scope("F1_fc1", _sid, False)
    _sid = nc.enter_named_scope("G2_gather", False)[0]
    # ---------------- AllGather fc1 ----------------
    cin6 = dram.tile([128, 4 * B], BF16)
    sync.dma_start(cin6[:], F1[:])
    g2 = dram.tile([N_CORES, 128, 4 * B], BF16, addr_space="Shared")
    pool_e.collective_compute(
        "AllGather", mybir.AluOpType.bypass,
        replica_groups=[list(range(N_CORES))],
        ins=[cin6.opt()], outs=[g2.opt()])
    p_f1.release()

    p_h2 = tc.alloc_tile_pool(name="p_h2", bufs=1, side="right")
    H2 = p_h2.tile([128, N_CORES * 4 * B], BF16)
    sync.dma_start(
        H2.rearrange("c (r f) -> c r f", r=N_CORES),
        g2.rearrange("r c f -> c r f"))

    nc.leave_named_scope("G2_gather", _sid, False)
    _sid = nc.enter_named_scope("F2_fc2", False)[0]
    # ---------------- fc2 ----------------
    p_f2 = tc.alloc_tile_pool(name="p_f2", bufs=1, side="left")
    F2 = p_f2.tile([128, 4 * B], BF16)
    fw2v = fw2.rearrange("k (a m) -> k a m", a=32)
    for mc in range(4):
        ps = psum.tile([128, B], F32, tag="ps")
        for kc in range(32):
            nc.tensor.matmul(
                ps[:], fw2v[:, kc, mc * 128:mc * 128 + 128],
                H2[:, kc * B:(kc + 1) * B],
                start=(kc == 0), stop=(kc == 31))
        act.activation(F2[:, mc * B:(mc + 1) * B], ps[:], RELU, bias=fb2[:, mc:mc + 1])
    if DEBUG_DUMPS:
        sync.dma_start(t["dF2"][:], F2[:])
    p_h2.release()

    nc.leave_named_scope("F2_fc2", _sid, False)
    _sid = nc.enter_named_scope("F3_fc3", False)[0]
    # ---------------- fc3 (partial over this core's 512 K) + AllReduce ----
    fw3v = fw3.rearrange("k (a m) -> k a m", a=4)
    ps = psum.tile([128, B], F32, tag="ps")
    for kc in range(4):
        nc.tensor.matmul(
            ps[0:100, :], fw3v[:, kc, :], F2[:, kc * B:(kc + 1) * B],
            start=(kc == 0), stop=(kc == 3))
    s3 = scr.tile([128, 512], F32, tag="ev3", bufs=1)
    act.activation(s3[0:100, 0:B], ps[0:100, :], IDENT, bias=fb3[:])  # + fb3/8
    cin7 = dram.tile([100, B], F32)
    sync.dma_start(cin7[:], s3[0:100, 0:B])
    g3 = dram.tile([100, B], F32)
    pool_e.collective_compute(
        "AllReduce", mybir.AluOpType.add,
        replica_groups=[list(range(N_CORES))],
        ins=[cin7.opt()], outs=[g3.opt()])
    sync.dma_start(yout[:], g3[:])
    nc.leave_named_scope("F3_fc3", _sid, False)
    p_f2.release()
    p_p5.release()
    p_fw2.release()
    p_w45.release()

    scr.release()
    dram.release()
    psum.release()


# ---------------------------------------------------------------------------
# host-side input prep (numpy; all weight arrays already in SBUF layout)
# ---------------------------------------------------------------------------

BF = ml_dtypes.bfloat16


def _prep_shared(w1, b1, w2, b2, w3, b3, w4, b4, w5, b5):
    f = np.float32
    # conv1: rows r = dyo*33 + dx*3 + c, row 99 = bias(ones); 4 passes dy=3p+dyo
    lw1 = np.zeros((100, 4 * 64), f)
    for p in range(4):
        for dyo in range(3):
            dy = 3 * p + dyo
            if dy > 10:
                continue
            for dx in range(11):
                for c in range(3):
                    lw1[dyo * 33 + dx * 3 + c, p * 64:(p + 1) * 64] = w1[:, c, dy, dx]
    lw1[99, 0:64] = b1
    # conv2: [128, (dy,j,m)]: j<3 -> rows s*64+c = ch c at dx=2j+s; j=3 -> dx=6
    lw2 = np.zeros((128, 7 * 4 * 192), f)
    for dy in range(7):
        for j in range(3):
            for s in range(2):
                lw2[s * 64:(s + 1) * 64, (dy * 4 + j) * 192:(dy * 4 + j + 1) * 192] = \
                    w2[:, :, dy, 2 * j + s].T
        lw2[0:64, (dy * 4 + 3) * 192:(dy * 4 + 4) * 192] = w2[:, :, dy, 6].T
    lb2 = np.zeros((128, 2), f)
    lb2[:, 0] = b2[0:128]
    lb2[0:64, 1] = b2[128:192]
    # conv3: [128, 15360]: cols blk*384+m (kc0, K=128); kc1 packs dx pairs
    # (0,1),(2,3),(4,) per dy at 9600+(dy*3+j)*384 — rows 0:64 = dx=2j,
    # rows 64:128 = dx=2j+1 (X3b partitions 64:128 hold x+1-shifted acts)
    lw3 = np.zeros((128, 15360), f)
    for dy in range(5):
        for dx in range(5):
            blk = dy * 5 + dx
            lw3[:, blk * 384:(blk + 1) * 384] = w3[:, 0:128, dy, dx].T
        for j in range(3):
            co = 9600 + (dy * 3 + j) * 384
            lw3[0:64, co:co + 384] = w3[:, 128:192, dy, 2 * j].T
            if j < 2:
                lw3[64:128, co:co + 384] = w3[:, 128:192, dy, 2 * j + 1].T
    lb3 = np.zeros((128, 3), f)
    lb3[:, 0] = b3[0:128]; lb3[:, 1] = b3[128:256]; lb3[:, 2] = b3[256:384]
    # conv4 / conv5: [128, (o, m)] with o = (dy*3+dx)*nkc + kc
    lw4 = np.zeros((128, 27 * 256), f)
    for dy in range(3):
        for dx in range(3):
            for kc in range(3):
                o = (dy * 3 + dx) * 3 + kc
                lw4[:, o * 256:(o + 1) * 256] = w4[:, kc * 128:(kc + 1) * 128, dy, dx].T
    lb4 = np.stack([b4[0:128], b4[128:256]], axis=1).astype(f)
    lw5 = np.zeros((128, 18 * 256), f)
    for dy in range(3):
        for dx in range(3):
            for kc in range(2):
                o = (dy * 3 + dx) * 2 + kc
                lw5[:, o * 256:(o + 1) * 256] = w5[:, kc * 128:(kc + 1) * 128, dy, dx].T
    lb5 = np.stack([b5[0:128], b5[128:256]], axis=1).astype(f)
    return dict(lw1=np.ascontiguousarray(lw1.astype(BF)),
                lw2=np.ascontiguousarray(lw2.astype(BF)), lb2=lb2,
                lw3=np.ascontiguousarray(lw3.astype(BF)), lb3=lb3,
                lw4=np.ascontiguousarray(lw4.astype(BF)), lb4=lb4,
                lw5=np.ascontiguousarray(lw5.astype(BF)), lb5=lb5)


def _prep_x13(x):
    """x [B,3,32,32] -> per-core [8, 100, 4*41*32] im2col-packed bf16."""
    f = np.float32
    xpad = np.zeros((B, 3, 44, 42), f)
    xpad[:, :, 5:37, 5:37] = x
    X = np.zeros((100, B, 41, 32), f)
    for dyo in range(3):
        for dx in range(11):
            for c in range(3):
                X[dyo * 33 + dx * 3 + c] = xpad[:, c, dyo:dyo + 41, dx:dx + 32]
    X[99] = 1.0
    out = []
    for r in range(N_CORES):
        pc = X[:, r * BC:(r + 1) * BC]  # [100, 32, 41, 32]
        pc = pc.reshape(100, 8, 4 * 41 * 32).transpose(1, 0, 2)
        out.append(np.ascontiguousarray(pc.astype(BF)))
    return out


def _prep_fc(fw1, fb1, fw2, fb2, fw3, fb3):
    f = np.float32
    outs = []
    for r in range(N_CORES):
        sl = slice(512 * r, 512 * (r + 1))
        # fw1s [128, (yx, cc, m)]: fw1[512r+m, (cc*128+k)*16+yx]
        fw1s = fw1[sl].reshape(512, 2, 128, 16).transpose(2, 3, 1, 0).reshape(128, -1)
        fb1s = fb1[sl].reshape(4, 128).T
        # fw2s [128, (kc, m)]: fw2[512r+m, kc*128+k]
        fw2s = fw2[sl].reshape(512, 32, 128).transpose(2, 1, 0).reshape(128, -1)
        fb2s = fb2[sl].reshape(4, 128).T
        # fw3s [128, (kc, m)]: fw3[m, 512r + kc*128 + k]
        fw3s = fw3[:, sl].reshape(100, 4, 128).transpose(2, 1, 0).reshape(128, -1)
        fb3s = (fb3 / N_CORES).reshape(100, 1)
        outs.append(dict(
            fw1s=np.ascontiguousarray(fw1s.astype(BF)),
            fb1s=np.ascontiguousarray(fb1s.astype(f)),
            fw2s=np.ascontiguousarray(fw2s.astype(BF)),
            fb2s=np.ascontiguousarray(fb2s.astype(f)),
            fw3s=np.ascontiguousarray(fw3s.astype(BF)),
            fb3s=np.ascontiguousarray(fb3s.astype(f)),
        ))
    return outs


_CACHE = {}

_SHAPES = dict(
    x13=(8, 100, 4 * 41 * 32), lw1=(100, 4 * 64),
    lw2=(128, 7 * 4 * 192), lb2=(128, 2),
    lw3=(128, 15360), lb3=(128, 3),
    lw4=(128, 27 * 256), lb4=(128, 2),
    lw5=(128, 18 * 256), lb5=(128, 2),
    fw1s=(128, 32 * 512), fb1s=(128, 4),
    fw2s=(128, 32 * 512), fb2s=(128, 4),
    fw3s=(128, 4 * 100), fb3s=(100, 1),
)

_BF16_INPUTS = {"x13", "lw1", "lw2", "lw3", "lw4", "lw5", "fw1s", "fw2s", "fw3s"}


def _build():
    if "nc" in _CACHE:
        return _CACHE["nc"]
    nc = bacc.Bacc("TRN2", target_bir_lowering=False, debug=False,
                   num_devices=N_CORES)

    def _dt(name):
        return BF16 if name in _BF16_INPUTS else F32
    t = {name: nc.dram_tensor(
            name, list(shape), _dt(name), kind="ExternalInput").ap()
         for name, shape in _SHAPES.items()}
    if DEBUG_DUMPS:
        for name, shape in _DBG_SHAPES.items():
            t[name] = nc.dram_tensor(
                name, list(shape), BF16, kind="ExternalOutput").ap()
    yout = nc.dram_tensor("yout", [100, B], F32, kind="ExternalOutput").ap()
    with tile.TileContext(nc) as tc:
        _emit(nc, tc, t, yout)
    nc.compile()
    _CACHE["nc"] = nc
    return nc


def kernel(x, w1, b1, w2, b2, w3, b3, w4, b4, w5, b5,
           fw1, fb1, fw2, fb2, fw3, fb3):
    args = [np.asarray(a, np.float32) for a in
            (x, w1, b1, w2, b2, w3, b3, w4, b4, w5, b5, fw1, fb1, fw2, fb2, fw3, fb3)]
    (x, w1, b1, w2, b2, w3, b3, w4, b4, w5, b5,
     fw1, fb1, fw2, fb2, fw3, fb3) = args
    nc = _build()
    shared = _prep_shared(w1, b1, w2, b2, w3, b3, w4, b4, w5, b5)
    x13s = _prep_x13(x)
    fcs = _prep_fc(fw1, fb1, fw2, fb2, fw3, fb3)
    in_maps = [{**shared, "x13": x13s[r], **fcs[r]} for r in range(N_CORES)]
    res = run_bass_kernel_spmd(nc, in_maps, list(range(N_CORES)))
    _CACHE["last_results"] = res.results
    y = res.results[0]["yout"]  # [100, 256]
    return np.ascontiguousarray(y.T)


# revision 30
# speedup vs baseline: 1.3714x; 1.0164x over previous
"""AlexNet-style CNN forward pass on 8 Trainium2 NeuronCores.

Strategy (v1):
  - Convs data-parallel: batch 256 -> 32 per core, channels on partitions,
    conv = sum of shifted matmuls over kernel offsets (weights replicated).
  - All conv weights/activations in bf16 (PSUM accumulates fp32); halves
    DMA + SBUF traffic and enables FWL weight loads.
  - Activations laid out batch-innermost ([ch, y, x, b]) so every matmul
    rhs is runs of 32 contiguous elements (the fp32 baseline's 8-elem
    strided runs slowed the PE rhs stream ~1.3x).
  - conv1 (cin=3) uses host-packed im2col rows (3 dy-offsets x 11 dx x 3 ch
    + ones row for fused bias -> K=100), 8 input chunks DMA'd on the sync
    queue (weights go on the scalar queue) so the first matmul starts ~8us
    in instead of 53us.
  - conv2 uses an x-shifted duplicate of its input (K=128 = 2 dx-offsets
    x 64 ch); conv3 likewise packs two dx offsets via a shifted copy.
  - FC layers model-parallel: each core owns 512 rows of fc1/fc2 and 512
    K-columns of fc3.  pool5 is AllGathered in 2 chunks that overlap fc1's
    K-accumulation into persistent PSUM; fc1 output AllGathered, fc3
    partials AllReduced.
"""

import os

import ml_dtypes
import numpy as np

import concourse.bass as bass
import concourse.mybir as mybir
import concourse.tile as tile
from concourse import bacc
from concourse.bass_utils import run_bass_kernel_spmd

N_CORES = 8
B = 256
BC = B // N_CORES  # 32 images per core

F32 = mybir.dt.float32
BF16 = mybir.dt.bfloat16
RELU = mybir.ActivationFunctionType.Relu
IDENT = mybir.ActivationFunctionType.Identity

DEBUG_DUMPS = os.environ.get("BASSDBG") == "1"

_DBG_SHAPES = dict(
    dX2s=(128, 22 * 23 * 32), dX3a=(128, 12 * 12 * 32), dX3b=(128, 12 * 12 * 32),
    dX4_0=(128, 3200), dX4_1=(128, 3200), dX4_2=(128, 3200),
    dX5_0=(128, 3200), dX5_1=(128, 3200),
    dP5_0=(128, 512), dP5_1=(128, 512),
    dF1=(128, 1024), dF2=(128, 1024),
)


def _emit(nc, tc, t, yout):
    """Emit the whole network. t: dict name -> DRAM AP."""
    sync = nc.sync
    act = nc.scalar
    dve = nc.vector
    pool_e = nc.gpsimd

    psum = tc.alloc_tile_pool(name="psum", bufs=6, space="PSUM")
    scr = tc.alloc_tile_pool(name="scr", bufs=1, side="left")
    dram = tc.alloc_tile_pool(name="dram", bufs=1, space="DRAM")

    # ---------------- phase pools ----------------
    p_w12 = tc.alloc_tile_pool(name="p_w12", bufs=1, side="left")
    p_x2s = tc.alloc_tile_pool(name="p_x2s", bufs=1, side="left")
    p_x13 = tc.alloc_tile_pool(name="p_x13", bufs=4, side="left")

    # x13 streams as 16 chunks of 2 images alternating between the two
    # HWDGE queues (sync + scalar); each queue sustains only ~100 GB/s.
    # conv1/conv2 weights interleave on the scalar queue.
    lw1 = p_w12.tile([100, 4 * 64], BF16)
    act.dma_start(lw1[:], t["lw1"][:])
    xts = []
    for ch in range(16):
        xt = p_x13.tile([100, 2 * 41 * 32], BF16, tag="x13")
        (sync if ch % 2 == 0 else act).dma_start(xt[:], t["x13"][ch])
        xts.append(xt)
        if ch == 5:
            lw2 = p_w12.tile([128, 7 * 4 * 192], BF16)
            act.dma_start(lw2[:], t["lw2"][:])
        if ch == 7:
            lb2 = p_w12.tile([128, 3], F32)
            act.dma_start(lb2[:], t["lb2"][:])

    # conv2 input: [128, 22y, 23x, 32b]; rows 0:64 ch c at x, rows 64:128
    # ch c at x+1 (b innermost so matmul rhs reads runs of 32 elems)
    X2s = p_x2s.tile([128, 22 * 23 * 32], BF16)
    pool_e.memset(X2s[:].bitcast(F32), 0.0)
    X2sv = X2s.rearrange("p (y x b) -> p y x b", y=22, x=23, b=32)

    # ---------------- conv1 + pool1 ----------------
    _sid = nc.enter_named_scope("L1_conv1", False)[0]
    for ch in range(16):
        xtv = xts[ch].rearrange("k (b y x) -> k b y x", b=2, y=41, x=32)
        for bl in range(2):
            b = ch * 2 + bl
            for h in range(2):  # vertical half of the 32x32 output
                ps = psum.tile([64, 512], F32, tag="ps")
                for pi in range(4):
                    p = 3 * pi
                    nc.tensor.matmul(
                        ps[:],
                        lw1[:, pi * 64:(pi + 1) * 64],
                        xtv[:, bl, h * 16 + p:h * 16 + p + 16, :],
                        start=(pi == 0), stop=(pi == 3),
                    )
                # evict+relu (bias came in via the ones-row), then 2x2 maxpool
                s1 = scr.tile([64, 512], BF16, tag="ev1", bufs=3)
                act.activation(s1[:], ps[:], RELU)
                sv = s1.rearrange("m (y x) -> m y x", y=16, x=32)
                m = scr.tile([64, 256], BF16, tag="m", bufs=4)
                mv = m.rearrange("m (y x) -> m y x", y=16, x=16)
                dve.tensor_max(mv, sv[:, :, 0::2], sv[:, :, 1::2])
                y0 = h * 8 + 3
                dve.tensor_max(
                    X2sv[0:64, y0:y0 + 8, 3:19, b],
                    mv[:, 0::2, :], mv[:, 1::2, :])
    # duplicate into the x+1-shifted partition block (b-inner: shift by 32
    # elems).  Flat-shifted copy in 4 slices alternating queues so conv2's
    # first rows can start early; wrapped elements land in dead pad columns.
    TOT = 22 * 23 * 32
    Q = TOT // 4
    for si in range(4):
        lo = si * Q
        hi = TOT - 32 if si == 3 else (si + 1) * Q
        (sync if si % 2 == 0 else act).dma_start(
            X2s[64:128, lo:hi], X2s[0:64, lo + 32:hi + 32])
    p_x13.release()
    nc.leave_named_scope("L1_conv1", _sid, False)

    # conv3 weights (prefetch during conv2) + conv3 input buffers
    p_w3 = tc.alloc_tile_pool(name="p_w3", bufs=1, side="right")
    p_x3 = tc.alloc_tile_pool(name="p_x3", bufs=1, side="right")
    lw3 = p_w3.tile([128, 15360], BF16)
    act.dma_start(lw3[:], t["lw3"][:])
    lb3 = p_w3.tile([128, 3], F32)
    act.dma_start(lb3[:], t["lb3"][:])
    # X3a: [128ch, 12y, 12x, 32b] (pad 2); X3b rows 0:64 = ch 128:192,
    # rows 64:128 its x+1-shifted copy (conv3 kc1 packs two dx per K=128)
    X3a = p_x3.tile([128, 12 * 12 * 32], BF16)
    X3b = p_x3.tile([128, 12 * 12 * 32], BF16)
    pool_e.memset(X3a[:].bitcast(F32), 0.0)
    pool_e.memset(X3b[:].bitcast(F32), 0.0)
    X3av = X3a.rearrange("p (y x b) -> p y x b", y=12, x=12, b=32)
    X3bv = X3b.rearrange("p (y x b) -> p y x b", y=12, x=12, b=32)

    # ---------------- conv2 + pool2 ----------------
    _sid = nc.enter_named_scope("L2_conv2", False)[0]
    lw2v = lw2.rearrange("k (a j m) -> k a j m", a=7, j=4, m=192)
    DJ = [(dy, j) for dy in range(7) for j in range(4)]

    def c2_rhs(y, dy, j):
        K = 128 if j < 3 else 64
        xoff = 2 * j if j < 3 else 6
        return X2sv[0:K, y + dy, xoff:xoff + 16, :]

    for yp in range(8):  # output row pairs
        ye, yo = 2 * yp, 2 * yp + 1
        # mc0 (M=128): one full-array psum tile per row
        s2keep = None
        for y in (ye, yo):
            ps = psum.tile([128, 512], F32, tag="ps")
            for i, (dy, j) in enumerate(DJ):
                K = 128 if j < 3 else 64
                nc.tensor.matmul(
                    ps[:], lw2v[0:K, dy, j, 0:128], c2_rhs(y, dy, j),
                    start=(i == 0), stop=(i == 27),
                )
            s2 = scr.tile([128, 512], BF16, tag="ev", bufs=4)
            act.activation(s2[:], ps[:], RELU, bias=lb2[:, 0:1])
            if y == ye:
                s2keep = s2
            else:
                pm = scr.tile([128, 512], BF16, tag="pm", bufs=2)
                dve.tensor_max(pm[:], s2keep[:], s2[:])
                pmv = pm.rearrange("m (x b) -> m x b", x=16, b=32)
                dve.tensor_max(X3av[0:128, yp + 2, 2:10, :],
                               pmv[:, 0::2, :], pmv[:, 1::2, :])
        # mc1 (M=64)
        s2keep = None
        for y in (ye, yo):
            ps = psum.tile([64, 512], F32, tag="ps")
            for i, (dy, j) in enumerate(DJ):
                K = 128 if j < 3 else 64
                nc.tensor.matmul(
                    ps[:], lw2v[0:K, dy, j, 128:192], c2_rhs(y, dy, j),
                    start=(i == 0), stop=(i == 27),
                )
            s2 = scr.tile([64, 512], BF16, tag="evb", bufs=4)
            act.activation(s2[:], ps[:], RELU, bias=lb2[0:64, 1:2])
            if y == ye:
                s2keep = s2
            else:
                pm = scr.tile([64, 512], BF16, tag="pmb", bufs=2)
                dve.tensor_max(pm[:], s2keep[:], s2[:])
                pmv = pm.rearrange("m (x b) -> m x b", x=16, b=32)
                dve.tensor_max(X3bv[0:64, yp + 2, 2:10, :],
                               pmv[:, 0::2, :], pmv[:, 1::2, :])
    # x+1-shifted duplicate for conv3's dx pairing (flat +32 shift)
    pool_e.dma_start(X3b[64:128, 0:12 * 12 * 32 - 32], X3b[0:64, 32:12 * 12 * 32])
    if DEBUG_DUMPS:
        sync.dma_start(t["dX2s"][:], X2s[:])
        sync.dma_start(t["dX3a"][:], X3a[:])
        sync.dma_start(t["dX3b"][:], X3b[:])
    nc.leave_named_scope("L2_conv2", _sid, False)
    p_x2s.release()
    p_w12.release()

    # conv4/5 weights (prefetch during conv3) + conv4 input buffers
    p_w45 = tc.alloc_tile_pool(name="p_w45", bufs=1, side="left")
    p_x4 = tc.alloc_tile_pool(name="p_x4", bufs=1, side="left")
    lw4 = p_w45.tile([128, 27 * 256], BF16)
    act.dma_start(lw4[:], t["lw4"][:])
    lb4 = p_w45.tile([128, 2], F32)
    act.dma_start(lb4[:], t["lb4"][:])
    lw5 = p_w45.tile([128, 18 * 256], BF16)
    act.dma_start(lw5[:], t["lw5"][:])
    lb5 = p_w45.tile([128, 2], F32)
    act.dma_start(lb5[:], t["lb5"][:])
    X4 = []
    X4v = []
    for i in range(3):
        X4.append(p_x4.tile([128, 10 * 10 * 32], BF16, name=f"X4_{i}"))
        pool_e.memset(X4[i][:].bitcast(F32), 0.0)
        X4v.append(X4[i].rearrange("p (y x b) -> p y x b", y=10, x=10, b=32))

    _sid = nc.enter_named_scope("L3_conv3", False)[0]
    # ---------------- conv3 ----------------
    for tt in range(4):  # output row pairs; psum free = 2y * 8x * 32b
        for mc in range(3):
            ps = psum.tile([128, 512], F32, tag="ps")
            first = True
            for dy in range(5):
                for dx in range(5):
                    blk = dy * 5 + dx
                    nc.tensor.matmul(
                        ps[:],
                        lw3[0:128, blk * 384 + mc * 128:blk * 384 + mc * 128 + 128],
                        X3av[0:128, 2 * tt + dy:2 * tt + dy + 2, dx:dx + 8, :],
                        start=first, stop=False,
                    )
                    first = False
                for j in range(3):  # kc1: dx pairs (0,1),(2,3),(4,)
                    K = 128 if j < 2 else 64
                    co = 9600 + (dy * 3 + j) * 384
                    nc.tensor.matmul(
                        ps[:],
                        lw3[0:K, co + mc * 128:co + mc * 128 + 128],
                        X3bv[0:K, 2 * tt + dy:2 * tt + dy + 2, 2 * j:2 * j + 8, :],
                        start=False, stop=(dy == 4 and j == 2),
                    )
            act.activation(
                X4v[mc][:, 2 * tt + 1:2 * tt + 3, 1:9, :],
                ps.rearrange("m (y x b) -> m y x b", y=2, x=8, b=32),
                RELU, bias=lb3[:, mc:mc + 1])
    if DEBUG_DUMPS:
        for i in range(3):
            sync.dma_start(t[f"dX4_{i}"][:], X4[i][:])
    nc.leave_named_scope("L3_conv3", _sid, False)
    p_x3.release()
    p_w3.release()

    # fc1 weights (prefetch during conv4) + conv5 input buffers
    p_fw1 = tc.alloc_tile_pool(name="p_fw1", bufs=1, side="right")
    p_x5 = tc.alloc_tile_pool(name="p_x5", bufs=1, side="right")
    fw1 = p_fw1.tile([128, 32 * 512], BF16)
    act.dma_start(fw1[:], t["fw1s"][:])
    fb1 = p_fw1.tile([128, 4], F32)
    act.dma_start(fb1[:], t["fb1s"][:])
    X5 = []
    X5v = []
    for i in range(2):
        X5.append(p_x5.tile([128, 10 * 10 * 32], BF16, name=f"X5_{i}"))
        pool_e.memset(X5[i][:].bitcast(F32), 0.0)
        X5v.append(X5[i].rearrange("p (y x b) -> p y x b", y=10, x=10, b=32))

    _sid = nc.enter_named_scope("L4_conv4", False)[0]
    # ---------------- conv4 ----------------
    lw4v = lw4.rearrange("k (o m) -> k o m", o=27)
    for tt in range(4):
        for mc in range(2):
            ps = psum.tile([128, 512], F32, tag="ps")
            first = True
            for dy in range(3):
                for dx in range(3):
                    for kc in range(3):
                        o = (dy * 3 + dx) * 3 + kc
                        nc.tensor.matmul(
                            ps[:],
                            lw4v[:, o, mc * 128:mc * 128 + 128],
                            X4v[kc][:, 2 * tt + dy:2 * tt + dy + 2, dx:dx + 8, :],
                            start=first, stop=(o == 26),
                        )
                        first = False
            act.activation(
                X5v[mc][:, 2 * tt + 1:2 * tt + 3, 1:9, :],
                ps.rearrange("m (y x b) -> m y x b", y=2, x=8, b=32),
                RELU, bias=lb4[:, mc:mc + 1])
    if DEBUG_DUMPS:
        for i in range(2):
            sync.dma_start(t[f"dX5_{i}"][:], X5[i][:])
    nc.leave_named_scope("L4_conv4", _sid, False)
    p_x4.release()

    # fc2/fc3 weights (DMA overlaps conv5 + the gathers + fc1)
    p_fw2 = tc.alloc_tile_pool(name="p_fw2", bufs=1, side="left")
    fw2 = p_fw2.tile([128, 32 * 512], BF16)
    act.dma_start(fw2[:], t["fw2s"][:])
    fb2 = p_fw2.tile([128, 4], F32)
    act.dma_start(fb2[:], t["fb2s"][:])
    fw3 = p_fw2.tile([128, 4 * 100], BF16)
    act.dma_start(fw3[:], t["fw3s"][:])
    fb3 = p_fw2.tile([100, 1], F32)
    act.dma_start(fb3[:], t["fb3s"][:])

    # pool5 output: [128ch, 4t, 4x, 32b] per channel half
    p_p5 = tc.alloc_tile_pool(name="p_p5", bufs=1, side="left")
    P5 = [p_p5.tile([128, BC * 16], BF16, name=f"P5_{i}") for i in range(2)]

    # fc1 persistent psum: 4 mc-quarter outputs of [128, 256] in 2 banks
    p_psF = tc.alloc_tile_pool(name="psumF", bufs=1, space="PSUM")
    psF = p_psF.tile([128, 1024], F32)

    # staging + gather buffers for the per-channel-half pool5 AllGather
    cin5 = [dram.tile([128, 512], BF16, name=f"cin5_{i}") for i in range(2)]
    g1 = [dram.tile([N_CORES, 128, 512], BF16, addr_space="Shared",
                    name=f"g1cc{i}") for i in range(2)]

    _sid = nc.enter_named_scope("L5_conv5", False)[0]
    # ---------------- conv5 + pool5 (mc-outer; gather per channel half) -----
    lw5v = lw5.rearrange("k (o m) -> k o m", o=18)
    for mc in range(2):
        for tt in range(4):
            ps = psum.tile([128, 512], F32, tag="ps")
            first = True
            for dy in range(3):
                for dx in range(3):
                    for kc in range(2):
                        o = (dy * 3 + dx) * 2 + kc
                        nc.tensor.matmul(
                            ps[:],
                            lw5v[:, o, mc * 128:mc * 128 + 128],
                            X5v[kc][:, 2 * tt + dy:2 * tt + dy + 2, dx:dx + 8, :],
                            start=first, stop=(o == 17),
                        )
                        first = False
            s5 = scr.tile([128, 512], BF16, tag="ev", bufs=4)
            act.activation(s5[:], ps[:], RELU, bias=lb5[:, mc:mc + 1])
            s5v = s5.rearrange("m (y x b) -> m y x b", y=2, x=8, b=32)
            pm = scr.tile([128, 256], BF16, tag="pm5", bufs=2)
            dve.tensor_max(pm[:], s5v[:, 0, :, :], s5v[:, 1, :, :])
            pmv = pm.rearrange("m (x b) -> m x b", x=8, b=32)
            p5v = P5[mc].rearrange("p (t x b) -> p t x b", t=4, x=4, b=32)
            dve.tensor_max(p5v[:, tt, :, :], pmv[:, 0::2, :], pmv[:, 1::2, :])
        # this channel half is complete: stage + gather while the other half
        # (or fc1's first chunk) computes
        sync.dma_start(cin5[mc][:], P5[mc][:])
        pool_e.collective_compute(
            "AllGather", mybir.AluOpType.bypass,
            replica_groups=[list(range(N_CORES))],
            ins=[cin5[mc].opt()], outs=[g1[mc].opt()])
    if DEBUG_DUMPS:
        for i in range(2):
            sync.dma_start(t[f"dP5_{i}"][:], P5[i][:])
    nc.leave_named_scope("L5_conv5", _sid, False)
    p_x5.release()

    _sid = nc.enter_named_scope("F1_fc1", False)[0]
    # ---------------- fc1: accumulate gathered channel halves into psF ------
    p_h1 = tc.alloc_tile_pool(name="p_h1", bufs=2, side="right")
    p_f1 = tc.alloc_tile_pool(name="p_f1", bufs=1, side="left")
    F1 = p_f1.tile([128, 4 * B], BF16)
    fw1v = fw1.rearrange("k (y c m) -> k y c m", y=16, c=2, m=512)
    for cc in range(2):  # gathered channel half
        H1 = p_h1.tile([128, 8 * 512], BF16, tag="h1")
        H1v = H1.rearrange("c (r t x b) -> c r t x b", r=8, t=4, x=4, b=32)
        H1f = H1.rearrange("c (r f) -> c r f", r=8)
        for rh in range(2):  # split the 1MB load across both HWDGE queues
            (sync if rh == 0 else act).dma_start(
                H1f[:, rh * 4:(rh + 1) * 4],
                g1[cc][rh * 4:(rh + 1) * 4].rearrange("r c f -> c r f"))
        for tt in range(4):
            for x in range(4):
                yx = tt * 4 + x
                for mq in range(4):
                    # start=True clears has_written for the WHOLE bank;
                    # mq pairs (0,1) and (2,3) share a bank, so only the
                    # bank's first matmul may set it — the partner region
                    # inits via overwrite-where-unset semantics.
                    nc.tensor.matmul(
                        psF[:, mq * 256:(mq + 1) * 256],
                        fw1v[:, yx, cc, mq * 128:mq * 128 + 128],
                        H1v[:, :, tt, x, :],
                        start=(cc == 0 and tt == 0 and x == 0 and mq % 2 == 0),
                        stop=(cc == 1 and tt == 3 and x == 3),
                        skip_group_check=True,
                    )
    for mq in range(4):
        act.activation(F1[:, mq * B:(mq + 1) * B],
                       psF[:, mq * 256:(mq + 1) * 256],
                       RELU, bias=fb1[:, mq:mq + 1])
    if DEBUG_DUMPS:
        sync.dma_start(t["dF1"][:], F1[:])
    p_h1.release()
    p_fw1.release()
    p_psF.release()

    nc.leave_named_scope("F1_fc1", _sid, False)
    _sid = nc.enter_named_scope("G2_gather", False)[0]
    # ---------------- AllGather fc1 ----------------
    cin6 = dram.tile([128, 4 * B], BF16)
    sync.dma_start(cin6[:], F1[:])
    g2 = dram.tile([N_CORES, 128, 4 * B], BF16, addr_space="Shared")
    pool_e.collective_compute(
        "AllGather", mybir.AluOpType.bypass,
        replica_groups=[list(range(N_CORES))],
        ins=[cin6.opt()], outs=[g2.opt()])
    p_f1.release()

    p_h2 = tc.alloc_tile_pool(name="p_h2", bufs=1, side="right")
    H2 = p_h2.tile([128, N_CORES * 4 * B], BF16)
    H2f = H2.rearrange("c (r f) -> c r f", r=N_CORES)
    for rh in range(2):  # split the 2MB load across both HWDGE queues
        (sync if rh == 0 else act).dma_start(
            H2f[:, rh * 4:(rh + 1) * 4],
            g2[rh * 4:(rh + 1) * 4].rearrange("r c f -> c r f"))

    nc.leave_named_scope("G2_gather", _sid, False)
    _sid = nc.enter_named_scope("F2_fc2", False)[0]
    # ---------------- fc2 ----------------
    p_f2 = tc.alloc_tile_pool(name="p_f2", bufs=1, side="left")
    F2 = p_f2.tile([128, 4 * B], BF16)
    fw2v = fw2.rearrange("k (a m) -> k a m", a=32)
    for mc in range(4):
        ps = psum.tile([128, B], F32, tag="ps")
        for kc in range(32):
            nc.tensor.matmul(
                ps[:], fw2v[:, kc, mc * 128:mc * 128 + 128],
                H2[:, kc * B:(kc + 1) * B],
                start=(kc == 0), stop=(kc == 31))
        act.activation(F2[:, mc * B:(mc + 1) * B], ps[:], RELU, bias=fb2[:, mc:mc + 1])
    if DEBUG_DUMPS:
        sync.dma_start(t["dF2"][:], F2[:])
    p_h2.release()

    nc.leave_named_scope("F2_fc2", _sid, False)
    _sid = nc.enter_named_scope("F3_fc3", False)[0]
    # ---------------- fc3 (partial over this core's 512 K) + AllReduce ----
    fw3v = fw3.rearrange("k (a m) -> k a m", a=4)
    ps = psum.tile([128, B], F32, tag="ps")
    for kc in range(4):
        nc.tensor.matmul(
            ps[0:100, :], fw3v[:, kc, :], F2[:, kc * B:(kc + 1) * B],
            start=(kc == 0), stop=(kc == 3))
    s3 = scr.tile([128, 512], BF16, tag="ev3", bufs=1)
    act.activation(s3[0:100, 0:B], ps[0:100, :], IDENT, bias=fb3[:])  # + fb3/8
    cin7 = dram.tile([100, B], BF16)
    sync.dma_start(cin7[:], s3[0:100, 0:B])
    g3 = dram.tile([100, B], BF16)
    pool_e.collective_compute(
        "AllReduce", mybir.AluOpType.add,
        replica_groups=[list(range(N_CORES))],
        ins=[cin7.opt()], outs=[g3.opt()])
    sync.dma_start(yout[:], g3[:])
    nc.leave_named_scope("F3_fc3", _sid, False)
    p_f2.release()
    p_p5.release()
    p_fw2.release()
    p_w45.release()

    scr.release()
    dram.release()
    psum.release()


# ---------------------------------------------------------------------------
# host-side input prep (numpy; all weight arrays already in SBUF layout)
# ---------------------------------------------------------------------------

BF = ml_dtypes.bfloat16


def _prep_shared(w1, b1, w2, b2, w3, b3, w4, b4, w5, b5):
    f = np.float32
    # conv1: rows r = dyo*33 + dx*3 + c, row 99 = bias(ones); 4 passes dy=3p+dyo
    lw1 = np.zeros((100, 4 * 64), f)
    for p in range(4):
        for dyo in range(3):
            dy = 3 * p + dyo
            if dy > 10:
                continue
            for dx in range(11):
                for c in range(3):
                    lw1[dyo * 33 + dx * 3 + c, p * 64:(p + 1) * 64] = w1[:, c, dy, dx]
    lw1[99, 0:64] = b1
    # conv2: [128, (dy,j,m)]: j<3 -> rows s*64+c = ch c at dx=2j+s; j=3 -> dx=6
    lw2 = np.zeros((128, 7 * 4 * 192), f)
    for dy in range(7):
        for j in range(3):
            for s in range(2):
                lw2[s * 64:(s + 1) * 64, (dy * 4 + j) * 192:(dy * 4 + j + 1) * 192] = \
                    w2[:, :, dy, 2 * j + s].T
        lw2[0:64, (dy * 4 + 3) * 192:(dy * 4 + 4) * 192] = w2[:, :, dy, 6].T
    # col 0: mc0 bias; col 2: mc1 bias duplicated onto both partition halves
    # (the col-tiled mc1 pair holds two output rows of the same 64 channels)
    lb2 = np.zeros((128, 3), f)
    lb2[:, 0] = b2[0:128]
    lb2[0:64, 1] = b2[128:192]
    lb2[0:64, 2] = b2[128:192]
    lb2[64:128, 2] = b2[128:192]
    # conv3: [128, 15360]: cols blk*384+m (kc0, K=128); kc1 packs dx pairs
    # (0,1),(2,3),(4,) per dy at 9600+(dy*3+j)*384 — rows 0:64 = dx=2j,
    # rows 64:128 = dx=2j+1 (X3b partitions 64:128 hold x+1-shifted acts)
    lw3 = np.zeros((128, 15360), f)
    for dy in range(5):
        for dx in range(5):
            blk = dy * 5 + dx
            lw3[:, blk * 384:(blk + 1) * 384] = w3[:, 0:128, dy, dx].T
        for j in range(3):
            co = 9600 + (dy * 3 + j) * 384
            lw3[0:64, co:co + 384] = w3[:, 128:192, dy, 2 * j].T
            if j < 2:
                lw3[64:128, co:co + 384] = w3[:, 128:192, dy, 2 * j + 1].T
    lb3 = np.zeros((128, 3), f)
    lb3[:, 0] = b3[0:128]; lb3[:, 1] = b3[128:256]; lb3[:, 2] = b3[256:384]
    # conv4 / conv5: [128, (o, m)] with o = (dy*3+dx)*nkc + kc
    lw4 = np.zeros((128, 27 * 256), f)
    for dy in range(3):
        for dx in range(3):
            for kc in range(3):
                o = (dy * 3 + dx) * 3 + kc
                lw4[:, o * 256:(o + 1) * 256] = w4[:, kc * 128:(kc + 1) * 128, dy, dx].T
    lb4 = np.stack([b4[0:128], b4[128:256]], axis=1).astype(f)
    lw5 = np.zeros((128, 18 * 256), f)
    for dy in range(3):
        for dx in range(3):
            for kc in range(2):
                o = (dy * 3 + dx) * 2 + kc
                lw5[:, o * 256:(o + 1) * 256] = w5[:, kc * 128:(kc + 1) * 128, dy, dx].T
    lb5 = np.stack([b5[0:128], b5[128:256]], axis=1).astype(f)
    return dict(lw1=np.ascontiguousarray(lw1.astype(BF)),
                lw2=np.ascontiguousarray(lw2.astype(BF)), lb2=lb2,
                lw3=np.ascontiguousarray(lw3.astype(BF)), lb3=lb3,
                lw4=np.ascontiguousarray(lw4.astype(BF)), lb4=lb4,
                lw5=np.ascontiguousarray(lw5.astype(BF)), lb5=lb5)


def _prep_x13(x):
    """x [B,3,32,32] -> per-core [16, 100, 2*41*32] im2col-packed bf16."""
    f = np.float32
    xpad = np.zeros((B, 3, 44, 42), f)
    xpad[:, :, 5:37, 5:37] = x
    X = np.zeros((100, B, 41, 32), f)
    for dyo in range(3):
        for dx in range(11):
            for c in range(3):
                X[dyo * 33 + dx * 3 + c] = xpad[:, c, dyo:dyo + 41, dx:dx + 32]
    X[99] = 1.0
    out = []
    for r in range(N_CORES):
        pc = X[:, r * BC:(r + 1) * BC]  # [100, 32, 41, 32]
        pc = pc.reshape(100, 16, 2 * 41 * 32).transpose(1, 0, 2)
        out.append(np.ascontiguousarray(pc.astype(BF)))
    return out


def _prep_fc(fw1, fb1, fw2, fb2, fw3, fb3):
    f = np.float32
    outs = []
    for r in range(N_CORES):
        sl = slice(512 * r, 512 * (r + 1))
        # fw1s [128, (yx, cc, m)]: fw1[512r+m, (cc*128+k)*16+yx]
        fw1s = fw1[sl].reshape(512, 2, 128, 16).transpose(2, 3, 1, 0).reshape(128, -1)
        fb1s = fb1[sl].reshape(4, 128).T
        # fw2s [128, (kc, m)]: fw2[512r+m, kc*128+k]
        fw2s = fw2[sl].reshape(512, 32, 128).transpose(2, 1, 0).reshape(128, -1)
        fb2s = fb2[sl].reshape(4, 128).T
        # fw3s [128, (kc, m)]: fw3[m, 512r + kc*128 + k]
        fw3s = fw3[:, sl].reshape(100, 4, 128).transpose(2, 1, 0).reshape(128, -1)
        fb3s = (fb3 / N_CORES).reshape(100, 1)
        outs.append(dict(
            fw1s=np.ascontiguousarray(fw1s.astype(BF)),
            fb1s=np.ascontiguousarray(fb1s.astype(f)),
            fw2s=np.ascontiguousarray(fw2s.astype(BF)),
            fb2s=np.ascontiguousarray(fb2s.astype(f)),
            fw3s=np.ascontiguousarray(fw3s.astype(BF)),
            fb3s=np.ascontiguousarray(fb3s.astype(f)),
        ))
    return outs


_CACHE = {}

_SHAPES = dict(
    x13=(16, 100, 2 * 41 * 32), lw1=(100, 4 * 64),
    lw2=(128, 7 * 4 * 192), lb2=(128, 3),
    lw3=(128, 15360), lb3=(128, 3),
    lw4=(128, 27 * 256), lb4=(128, 2),
    lw5=(128, 18 * 256), lb5=(128, 2),
    fw1s=(128, 32 * 512), fb1s=(128, 4),
    fw2s=(128, 32 * 512), fb2s=(128, 4),
    fw3s=(128, 4 * 100), fb3s=(100, 1),
)

_BF16_INPUTS = {"x13", "lw1", "lw2", "lw3", "lw4", "lw5", "fw1s", "fw2s", "fw3s"}


def _build():
    if "nc" in _CACHE:
        return _CACHE["nc"]
    nc = bacc.Bacc("TRN2", target_bir_lowering=False, debug=False,
                   num_devices=N_CORES)

    def _dt(name):
        return BF16 if name in _BF16_INPUTS else F32
    t = {name: nc.dram_tensor(
            name, list(shape), _dt(name), kind="ExternalInput").ap()
         for name, shape in _SHAPES.items()}
    if DEBUG_DUMPS:
        for name, shape in _DBG_SHAPES.items():
            t[name] = nc.dram_tensor(
                name, list(shape), BF16, kind="ExternalOutput").ap()
    yout = nc.dram_tensor("yout", [100, B], BF16, kind="ExternalOutput").ap()
    with tile.TileContext(nc) as tc:
        _emit(nc, tc, t, yout)
    nc.compile()
    _CACHE["nc"] = nc
    return nc


def kernel(x, w1, b1, w2, b2, w3, b3, w4, b4, w5, b5,
           fw1, fb1, fw2, fb2, fw3, fb3):
    args = [np.asarray(a, np.float32) for a in
            (x, w1, b1, w2, b2, w3, b3, w4, b4, w5, b5, fw1, fb1, fw2, fb2, fw3, fb3)]
    (x, w1, b1, w2, b2, w3, b3, w4, b4, w5, b5,
     fw1, fb1, fw2, fb2, fw3, fb3) = args
    nc = _build()
    shared = _prep_shared(w1, b1, w2, b2, w3, b3, w4, b4, w5, b5)
    x13s = _prep_x13(x)
    fcs = _prep_fc(fw1, fb1, fw2, fb2, fw3, fb3)
    in_maps = [{**shared, "x13": x13s[r], **fcs[r]} for r in range(N_CORES)]
    res = run_bass_kernel_spmd(nc, in_maps, list(range(N_CORES)))
    _CACHE["last_results"] = res.results
    y = res.results[0]["yout"]  # [100, 256] bf16
    return np.ascontiguousarray(y.T.astype(np.float32))


# revision 35
# speedup vs baseline: 1.4061x; 1.0253x over previous
"""AlexNet-style CNN forward pass on 8 Trainium2 NeuronCores.

Strategy (v1):
  - Convs data-parallel: batch 256 -> 32 per core, channels on partitions,
    conv = sum of shifted matmuls over kernel offsets (weights replicated).
  - All conv weights/activations in bf16 (PSUM accumulates fp32); halves
    DMA + SBUF traffic and enables FWL weight loads.
  - Activations laid out batch-innermost ([ch, y, x, b]) so every matmul
    rhs is runs of 32 contiguous elements (the fp32 baseline's 8-elem
    strided runs slowed the PE rhs stream ~1.3x).
  - conv1 (cin=3) uses host-packed im2col rows (3 dy-offsets x 11 dx x 3 ch
    + ones row for fused bias -> K=100), 8 input chunks DMA'd on the sync
    queue (weights go on the scalar queue) so the first matmul starts ~8us
    in instead of 53us.
  - conv2 uses an x-shifted duplicate of its input (K=128 = 2 dx-offsets
    x 64 ch); conv3 likewise packs two dx offsets via a shifted copy.
  - FC layers model-parallel: each core owns 512 rows of fc1/fc2 and 512
    K-columns of fc3.  pool5 is AllGathered in 2 chunks that overlap fc1's
    K-accumulation into persistent PSUM; fc1 output AllGathered, fc3
    partials AllReduced.
"""

import os

import ml_dtypes
import numpy as np

import concourse.bass as bass
import concourse.mybir as mybir
import concourse.tile as tile
from concourse import bacc
from concourse.bass_utils import run_bass_kernel_spmd

N_CORES = 8
B = 256
BC = B // N_CORES  # 32 images per core

F32 = mybir.dt.float32
BF16 = mybir.dt.bfloat16
RELU = mybir.ActivationFunctionType.Relu
IDENT = mybir.ActivationFunctionType.Identity

DEBUG_DUMPS = os.environ.get("BASSDBG") == "1"

_DBG_SHAPES = dict(
    dX2s=(128, 22 * 23 * 32), dX3a=(128, 12 * 12 * 32), dX3b=(128, 12 * 12 * 32),
    dX4_0=(128, 3200), dX4_1=(128, 3200), dX4_2=(128, 3200),
    dX5_0=(128, 3200), dX5_1=(128, 3200),
    dP5_0=(128, 512), dP5_1=(128, 512),
    dF1=(128, 1024), dF2=(128, 1024),
)


def _emit(nc, tc, t, yout):
    """Emit the whole network. t: dict name -> DRAM AP."""
    sync = nc.sync
    act = nc.scalar
    dve = nc.vector
    pool_e = nc.gpsimd

    psum = tc.alloc_tile_pool(name="psum", bufs=6, space="PSUM")
    scr = tc.alloc_tile_pool(name="scr", bufs=1, side="left")
    dram = tc.alloc_tile_pool(name="dram", bufs=1, space="DRAM")

    # ---------------- phase pools ----------------
    p_w12 = tc.alloc_tile_pool(name="p_w12", bufs=1, side="left")
    p_x2s = tc.alloc_tile_pool(name="p_x2s", bufs=1, side="left")
    p_x13 = tc.alloc_tile_pool(name="p_x13", bufs=4, side="left")

    # x13 streams as 16 chunks of 2 images alternating between the two
    # HWDGE queues (sync + scalar); each queue sustains only ~100 GB/s.
    # conv1/conv2 weights interleave on the scalar queue.
    lw1 = p_w12.tile([100, 4 * 64], BF16)
    act.dma_start(lw1[:], t["lw1"][:])
    xts = []
    for ch in range(16):
        xt = p_x13.tile([100, 2 * 41 * 32], BF16, tag="x13")
        (sync if ch % 2 == 0 else act).dma_start(xt[:], t["x13"][ch])
        xts.append(xt)
        if ch == 5:
            lw2 = p_w12.tile([128, 7 * 4 * 192], BF16)
            act.dma_start(lw2[:], t["lw2"][:])
        if ch == 7:
            lb2 = p_w12.tile([128, 3], F32)
            act.dma_start(lb2[:], t["lb2"][:])

    # conv2 input: [128, 22y, 23x, 32b]; rows 0:64 ch c at x, rows 64:128
    # ch c at x+1 (b innermost so matmul rhs reads runs of 32 elems)
    X2s = p_x2s.tile([128, 22 * 23 * 32], BF16)
    pool_e.memset(X2s[:].bitcast(F32), 0.0)
    X2sv = X2s.rearrange("p (y x b) -> p y x b", y=22, x=23, b=32)

    # ---------------- conv1 + pool1 ----------------
    _sid = nc.enter_named_scope("L1_conv1", False)[0]
    for ch in range(16):
        xtv = xts[ch].rearrange("k (b y x) -> k b y x", b=2, y=41, x=32)
        for bl in range(2):
            b = ch * 2 + bl
            for h in range(2):  # vertical half of the 32x32 output
                ps = psum.tile([64, 512], F32, tag="ps")
                for pi in range(4):
                    p = 3 * pi
                    nc.tensor.matmul(
                        ps[:],
                        lw1[:, pi * 64:(pi + 1) * 64],
                        xtv[:, bl, h * 16 + p:h * 16 + p + 16, :],
                        start=(pi == 0), stop=(pi == 3),
                    )
                # evict+relu (bias came in via the ones-row), then 2x2 maxpool
                s1 = scr.tile([64, 512], BF16, tag="ev1", bufs=3)
                act.activation(s1[:], ps[:], RELU)
                sv = s1.rearrange("m (y x) -> m y x", y=16, x=32)
                m = scr.tile([64, 256], BF16, tag="m", bufs=4)
                mv = m.rearrange("m (y x) -> m y x", y=16, x=16)
                dve.tensor_max(mv, sv[:, :, 0::2], sv[:, :, 1::2])
                y0 = h * 8 + 3
                dve.tensor_max(
                    X2sv[0:64, y0:y0 + 8, 3:19, b],
                    mv[:, 0::2, :], mv[:, 1::2, :])
    # duplicate into the x+1-shifted partition block (b-inner: shift by 32
    # elems).  Flat-shifted copy in 3 slices on 3 DMA paths so conv2's
    # first rows can start early; wrapped elements land in dead pad columns.
    TOT = 22 * 23 * 32
    cuts = [0, TOT // 3, 2 * TOT // 3, TOT - 32]
    for si, eng in enumerate((sync, act, pool_e)):
        lo, hi = cuts[si], cuts[si + 1]
        eng.dma_start(X2s[64:128, lo:hi], X2s[0:64, lo + 32:hi + 32])
    p_x13.release()
    nc.leave_named_scope("L1_conv1", _sid, False)

    # conv3 weights (prefetch during conv2) + conv3 input buffers
    p_w3 = tc.alloc_tile_pool(name="p_w3", bufs=1, side="right")
    p_x3 = tc.alloc_tile_pool(name="p_x3", bufs=1, side="right")
    lw3 = p_w3.tile([128, 15360], BF16)
    act.dma_start(lw3[:], t["lw3"][:])
    lb3 = p_w3.tile([128, 3], F32)
    act.dma_start(lb3[:], t["lb3"][:])
    # X3a: [128ch, 12y, 12x, 32b] (pad 2); X3b rows 0:64 = ch 128:192,
    # rows 64:128 its x+1-shifted copy (conv3 kc1 packs two dx per K=128)
    X3a = p_x3.tile([128, 12 * 12 * 32], BF16)
    X3b = p_x3.tile([128, 12 * 12 * 32], BF16)
    pool_e.memset(X3a[:].bitcast(F32), 0.0)
    pool_e.memset(X3b[:].bitcast(F32), 0.0)
    X3av = X3a.rearrange("p (y x b) -> p y x b", y=12, x=12, b=32)
    X3bv = X3b.rearrange("p (y x b) -> p y x b", y=12, x=12, b=32)

    # ---------------- conv2 + pool2 ----------------
    _sid = nc.enter_named_scope("L2_conv2", False)[0]
    lw2v = lw2.rearrange("k (a j m) -> k a j m", a=7, j=4, m=192)
    DJ = [(dy, j) for dy in range(7) for j in range(4)]

    def c2_rhs(y, dy, j):
        K = 128 if j < 3 else 64
        xoff = 2 * j if j < 3 else 6
        return X2sv[0:K, y + dy, xoff:xoff + 16, :]

    for yp in range(8):  # output row pairs
        ye, yo = 2 * yp, 2 * yp + 1
        # mc0 (M=128): one full-array psum tile per row
        s2keep = None
        for y in (ye, yo):
            ps = psum.tile([128, 512], F32, tag="ps")
            for i, (dy, j) in enumerate(DJ):
                K = 128 if j < 3 else 64
                nc.tensor.matmul(
                    ps[:], lw2v[0:K, dy, j, 0:128], c2_rhs(y, dy, j),
                    start=(i == 0), stop=(i == 27),
                )
            s2 = scr.tile([128, 512], BF16, tag="ev", bufs=4)
            act.activation(s2[:], ps[:], RELU, bias=lb2[:, 0:1])
            if y == ye:
                s2keep = s2
            else:
                pm = scr.tile([128, 512], BF16, tag="pm", bufs=2)
                dve.tensor_max(pm[:], s2keep[:], s2[:])
                pmv = pm.rearrange("m (x b) -> m x b", x=16, b=32)
                dve.tensor_max(X3av[0:128, yp + 2, 2:10, :],
                               pmv[:, 0::2, :], pmv[:, 1::2, :])
        # mc1 (M=64)
        s2keep = None
        for y in (ye, yo):
            ps = psum.tile([64, 512], F32, tag="ps")
            for i, (dy, j) in enumerate(DJ):
                K = 128 if j < 3 else 64
                nc.tensor.matmul(
                    ps[:], lw2v[0:K, dy, j, 128:192], c2_rhs(y, dy, j),
                    start=(i == 0), stop=(i == 27),
                )
            s2 = scr.tile([64, 512], BF16, tag="evb", bufs=4)
            act.activation(s2[:], ps[:], RELU, bias=lb2[0:64, 1:2])
            if y == ye:
                s2keep = s2
            else:
                pm = scr.tile([64, 512], BF16, tag="pmb", bufs=2)
                dve.tensor_max(pm[:], s2keep[:], s2[:])
                pmv = pm.rearrange("m (x b) -> m x b", x=16, b=32)
                dve.tensor_max(X3bv[0:64, yp + 2, 2:10, :],
                               pmv[:, 0::2, :], pmv[:, 1::2, :])
    # x+1-shifted duplicate for conv3's dx pairing (flat +32 shift)
    X3TOT = 12 * 12 * 32
    sync.dma_start(X3b[64:128, 0:X3TOT // 2], X3b[0:64, 32:X3TOT // 2 + 32])
    act.dma_start(X3b[64:128, X3TOT // 2:X3TOT - 32], X3b[0:64, X3TOT // 2 + 32:X3TOT])
    if DEBUG_DUMPS:
        sync.dma_start(t["dX2s"][:], X2s[:])
        sync.dma_start(t["dX3a"][:], X3a[:])
        sync.dma_start(t["dX3b"][:], X3b[:])
    nc.leave_named_scope("L2_conv2", _sid, False)
    p_x2s.release()
    p_w12.release()

    # conv4/5 weights (prefetch during conv3) + conv4 input buffers
    p_w45 = tc.alloc_tile_pool(name="p_w45", bufs=1, side="left")
    p_x4 = tc.alloc_tile_pool(name="p_x4", bufs=1, side="left")
    lw4 = p_w45.tile([128, 27 * 256], BF16)
    act.dma_start(lw4[:], t["lw4"][:])
    lb4 = p_w45.tile([128, 2], F32)
    act.dma_start(lb4[:], t["lb4"][:])
    lw5 = p_w45.tile([128, 18 * 256], BF16)
    act.dma_start(lw5[:], t["lw5"][:])
    lb5 = p_w45.tile([128, 2], F32)
    act.dma_start(lb5[:], t["lb5"][:])
    X4 = []
    X4v = []
    for i in range(3):
        X4.append(p_x4.tile([128, 10 * 10 * 32], BF16, name=f"X4_{i}"))
        pool_e.memset(X4[i][:].bitcast(F32), 0.0)
        X4v.append(X4[i].rearrange("p (y x b) -> p y x b", y=10, x=10, b=32))

    _sid = nc.enter_named_scope("L3_conv3", False)[0]
    # ---------------- conv3 ----------------
    for tt in range(4):  # output row pairs; psum free = 2y * 8x * 32b
        for mc in range(3):
            ps = psum.tile([128, 512], F32, tag="ps")
            first = True
            for dy in range(5):
                for dx in range(5):
                    blk = dy * 5 + dx
                    nc.tensor.matmul(
                        ps[:],
                        lw3[0:128, blk * 384 + mc * 128:blk * 384 + mc * 128 + 128],
                        X3av[0:128, 2 * tt + dy:2 * tt + dy + 2, dx:dx + 8, :],
                        start=first, stop=False,
                    )
                    first = False
                for j in range(3):  # kc1: dx pairs (0,1),(2,3),(4,)
                    K = 128 if j < 2 else 64
                    co = 9600 + (dy * 3 + j) * 384
                    nc.tensor.matmul(
                        ps[:],
                        lw3[0:K, co + mc * 128:co + mc * 128 + 128],
                        X3bv[0:K, 2 * tt + dy:2 * tt + dy + 2, 2 * j:2 * j + 8, :],
                        start=False, stop=(dy == 4 and j == 2),
                    )
            act.activation(
                X4v[mc][:, 2 * tt + 1:2 * tt + 3, 1:9, :],
                ps.rearrange("m (y x b) -> m y x b", y=2, x=8, b=32),
                RELU, bias=lb3[:, mc:mc + 1])
    if DEBUG_DUMPS:
        for i in range(3):
            sync.dma_start(t[f"dX4_{i}"][:], X4[i][:])
    nc.leave_named_scope("L3_conv3", _sid, False)
    p_x3.release()
    p_w3.release()

    # fc1 weights (prefetch during conv4) + conv5 input buffers
    p_fw1 = tc.alloc_tile_pool(name="p_fw1", bufs=1, side="right")
    p_x5 = tc.alloc_tile_pool(name="p_x5", bufs=1, side="right")
    fw1 = p_fw1.tile([128, 32 * 512], BF16)
    act.dma_start(fw1[:], t["fw1s"][:])
    fb1 = p_fw1.tile([128, 4], F32)
    act.dma_start(fb1[:], t["fb1s"][:])
    X5 = []
    X5v = []
    for i in range(2):
        X5.append(p_x5.tile([128, 10 * 10 * 32], BF16, name=f"X5_{i}"))
        pool_e.memset(X5[i][:].bitcast(F32), 0.0)
        X5v.append(X5[i].rearrange("p (y x b) -> p y x b", y=10, x=10, b=32))

    _sid = nc.enter_named_scope("L4_conv4", False)[0]
    # ---------------- conv4 ----------------
    lw4v = lw4.rearrange("k (o m) -> k o m", o=27)
    for tt in range(4):
        for mc in range(2):
            ps = psum.tile([128, 512], F32, tag="ps")
            first = True
            for dy in range(3):
                for dx in range(3):
                    for kc in range(3):
                        o = (dy * 3 + dx) * 3 + kc
                        nc.tensor.matmul(
                            ps[:],
                            lw4v[:, o, mc * 128:mc * 128 + 128],
                            X4v[kc][:, 2 * tt + dy:2 * tt + dy + 2, dx:dx + 8, :],
                            start=first, stop=(o == 26),
                        )
                        first = False
            act.activation(
                X5v[mc][:, 2 * tt + 1:2 * tt + 3, 1:9, :],
                ps.rearrange("m (y x b) -> m y x b", y=2, x=8, b=32),
                RELU, bias=lb4[:, mc:mc + 1])
    if DEBUG_DUMPS:
        for i in range(2):
            sync.dma_start(t[f"dX5_{i}"][:], X5[i][:])
    nc.leave_named_scope("L4_conv4", _sid, False)
    p_x4.release()

    # fc2/fc3 weights (DMA overlaps conv5 + the gathers + fc1)
    p_fw2 = tc.alloc_tile_pool(name="p_fw2", bufs=1, side="left")
    fw2 = p_fw2.tile([128, 32 * 512], BF16)
    act.dma_start(fw2[:], t["fw2s"][:])
    fb2 = p_fw2.tile([128, 4], F32)
    act.dma_start(fb2[:], t["fb2s"][:])
    fw3 = p_fw2.tile([128, 4 * 100], BF16)
    act.dma_start(fw3[:], t["fw3s"][:])
    fb3 = p_fw2.tile([100, 1], F32)
    act.dma_start(fb3[:], t["fb3s"][:])

    # pool5 output: [128ch, 4t, 4x, 32b] per channel half
    p_p5 = tc.alloc_tile_pool(name="p_p5", bufs=1, side="left")
    P5 = [p_p5.tile([128, BC * 16], BF16, name=f"P5_{i}") for i in range(2)]

    # fc1 persistent psum: 4 mc-quarter outputs of [128, 256] in 2 banks
    p_psF = tc.alloc_tile_pool(name="psumF", bufs=1, space="PSUM")
    psF = p_psF.tile([128, 1024], F32)

    # staging + gather buffers for the per-channel-half pool5 AllGather
    cin5 = [dram.tile([128, 512], BF16, name=f"cin5_{i}") for i in range(2)]
    g1 = [dram.tile([N_CORES, 128, 512], BF16, addr_space="Shared",
                    name=f"g1cc{i}") for i in range(2)]

    # tiny warm-up collective: the first real gather after a long CC-idle
    # stretch pays ~11us of trigger latency; this keeps the CC path warm
    cw_in = dram.tile([128, 8], BF16)
    cw_out = dram.tile([N_CORES, 128, 8], BF16, addr_space="Shared")

    _sid = nc.enter_named_scope("L5_conv5", False)[0]
    # ---------------- conv5 + pool5 (mc-outer; gather per channel half) -----
    lw5v = lw5.rearrange("k (o m) -> k o m", o=18)
    for mc in range(2):
        for tt in range(4):
            ps = psum.tile([128, 512], F32, tag="ps")
            first = True
            for dy in range(3):
                for dx in range(3):
                    for kc in range(2):
                        o = (dy * 3 + dx) * 2 + kc
                        nc.tensor.matmul(
                            ps[:],
                            lw5v[:, o, mc * 128:mc * 128 + 128],
                            X5v[kc][:, 2 * tt + dy:2 * tt + dy + 2, dx:dx + 8, :],
                            start=first, stop=(o == 17),
                        )
                        first = False
            s5 = scr.tile([128, 512], BF16, tag="ev", bufs=4)
            act.activation(s5[:], ps[:], RELU, bias=lb5[:, mc:mc + 1])
            s5v = s5.rearrange("m (y x b) -> m y x b", y=2, x=8, b=32)
            pm = scr.tile([128, 256], BF16, tag="pm5", bufs=2)
            dve.tensor_max(pm[:], s5v[:, 0, :, :], s5v[:, 1, :, :])
            pmv = pm.rearrange("m (x b) -> m x b", x=8, b=32)
            p5v = P5[mc].rearrange("p (t x b) -> p t x b", t=4, x=4, b=32)
            dve.tensor_max(p5v[:, tt, :, :], pmv[:, 0::2, :], pmv[:, 1::2, :])
            if mc == 0 and tt == 1:
                sync.dma_start(cw_in[:], lw5[:, 0:8])
                pool_e.collective_compute(
                    "AllGather", mybir.AluOpType.bypass,
                    replica_groups=[list(range(N_CORES))],
                    ins=[cw_in.opt()], outs=[cw_out.opt()])
        # this channel half is complete: stage + gather while the other half
        # (or fc1's first chunk) computes
        sync.dma_start(cin5[mc][:], P5[mc][:])
        pool_e.collective_compute(
            "AllGather", mybir.AluOpType.bypass,
            replica_groups=[list(range(N_CORES))],
            ins=[cin5[mc].opt()], outs=[g1[mc].opt()])
    if DEBUG_DUMPS:
        for i in range(2):
            sync.dma_start(t[f"dP5_{i}"][:], P5[i][:])
    nc.leave_named_scope("L5_conv5", _sid, False)
    p_x5.release()

    _sid = nc.enter_named_scope("F1_fc1", False)[0]
    # ---------------- fc1: accumulate gathered channel halves into psF ------
    p_h1 = tc.alloc_tile_pool(name="p_h1", bufs=2, side="right")
    p_f1 = tc.alloc_tile_pool(name="p_f1", bufs=1, side="left")
    F1 = p_f1.tile([128, 4 * B], BF16)
    fw1v = fw1.rearrange("k (y c m) -> k y c m", y=16, c=2, m=512)
    for cc in range(2):  # gathered channel half
        H1 = p_h1.tile([128, 8 * 512], BF16, tag="h1")
        H1v = H1.rearrange("c (r t x b) -> c r t x b", r=8, t=4, x=4, b=32)
        H1f = H1.rearrange("c (r f) -> c r f", r=8)
        for rh in range(2):  # split the 1MB load across both HWDGE queues
            (sync if rh == 0 else act).dma_start(
                H1f[:, rh * 4:(rh + 1) * 4],
                g1[cc][rh * 4:(rh + 1) * 4].rearrange("r c f -> c r f"))
        for tt in range(4):
            for x in range(4):
                yx = tt * 4 + x
                for mq in range(4):
                    # start=True clears has_written for the WHOLE bank;
                    # mq pairs (0,1) and (2,3) share a bank, so only the
                    # bank's first matmul may set it — the partner region
                    # inits via overwrite-where-unset semantics.
                    nc.tensor.matmul(
                        psF[:, mq * 256:(mq + 1) * 256],
                        fw1v[:, yx, cc, mq * 128:mq * 128 + 128],
                        H1v[:, :, tt, x, :],
                        start=(cc == 0 and tt == 0 and x == 0 and mq % 2 == 0),
                        stop=(cc == 1 and tt == 3 and x == 3),
                        skip_group_check=True,
                    )
    # evict + stage + gather fc1 output in two halves so the second gather
    # overlaps fc2's first-half accumulation
    cin6 = [dram.tile([128, 2 * B], BF16, name=f"cin6_{h}") for h in range(2)]
    g2h = [dram.tile([N_CORES, 128, 2 * B], BF16, addr_space="Shared",
                     name=f"g2_{h}") for h in range(2)]
    for half in range(2):
        for mq in (2 * half, 2 * half + 1):
            act.activation(F1[:, mq * B:(mq + 1) * B],
                           psF[:, mq * 256:(mq + 1) * 256],
                           RELU, bias=fb1[:, mq:mq + 1])
        sync.dma_start(cin6[half][:], F1[:, half * 2 * B:(half + 1) * 2 * B])
        pool_e.collective_compute(
            "AllGather", mybir.AluOpType.bypass,
            replica_groups=[list(range(N_CORES))],
            ins=[cin6[half].opt()], outs=[g2h[half].opt()])
    if DEBUG_DUMPS:
        sync.dma_start(t["dF1"][:], F1[:])
    p_h1.release()
    p_fw1.release()
    p_psF.release()

    nc.leave_named_scope("F1_fc1", _sid, False)
    _sid = nc.enter_named_scope("G2_gather", False)[0]
    # ---------------- load gathered fc1 ----------------
    p_h2 = tc.alloc_tile_pool(name="p_h2", bufs=1, side="right")
    H2 = p_h2.tile([128, N_CORES * 4 * B], BF16)
    H2v = H2.rearrange("c (r h f) -> c r h f", r=N_CORES, h=2)
    for half in range(2):
        for rh in range(2):  # split each 1MB load across both HWDGE queues
            (sync if rh == 0 else act).dma_start(
                H2v[:, rh * 4:(rh + 1) * 4, half],
                g2h[half][rh * 4:(rh + 1) * 4].rearrange("r c f -> c r f"))
    p_f1.release()

    nc.leave_named_scope("G2_gather", _sid, False)
    _sid = nc.enter_named_scope("F2_fc2", False)[0]
    # ---------------- fc2 (half-0 kc blocks first: they arrive first) ------
    p_f2 = tc.alloc_tile_pool(name="p_f2", bufs=1, side="left")
    F2 = p_f2.tile([128, 4 * B], BF16)
    fw2v = fw2.rearrange("k (a m) -> k a m", a=32)
    kcs = [r * 4 + 2 * half + q
           for half in range(2) for r in range(8) for q in range(2)]
    for mc in range(4):
        ps = psum.tile([128, B], F32, tag="ps")
        for i, kc in enumerate(kcs):
            nc.tensor.matmul(
                ps[:], fw2v[:, kc, mc * 128:mc * 128 + 128],
                H2[:, kc * B:(kc + 1) * B],
                start=(i == 0), stop=(i == 31))
        act.activation(F2[:, mc * B:(mc + 1) * B], ps[:], RELU, bias=fb2[:, mc:mc + 1])
    if DEBUG_DUMPS:
        sync.dma_start(t["dF2"][:], F2[:])
    p_h2.release()

    nc.leave_named_scope("F2_fc2", _sid, False)
    _sid = nc.enter_named_scope("F3_fc3", False)[0]
    # ---------------- fc3 (partial over this core's 512 K) + AllReduce ----
    fw3v = fw3.rearrange("k (a m) -> k a m", a=4)
    ps = psum.tile([128, B], F32, tag="ps")
    for kc in range(4):
        nc.tensor.matmul(
            ps[0:100, :], fw3v[:, kc, :], F2[:, kc * B:(kc + 1) * B],
            start=(kc == 0), stop=(kc == 3))
    s3 = scr.tile([128, 512], BF16, tag="ev3", bufs=1)
    act.activation(s3[0:100, 0:B], ps[0:100, :], IDENT, bias=fb3[:])  # + fb3/8
    cin7 = dram.tile([100, B], BF16)
    sync.dma_start(cin7[:], s3[0:100, 0:B])
    g3 = dram.tile([100, B], BF16)
    pool_e.collective_compute(
        "AllReduce", mybir.AluOpType.add,
        replica_groups=[list(range(N_CORES))],
        ins=[cin7.opt()], outs=[g3.opt()])
    sync.dma_start(yout[:], g3[:])
    nc.leave_named_scope("F3_fc3", _sid, False)
    p_f2.release()
    p_p5.release()
    p_fw2.release()
    p_w45.release()

    scr.release()
    dram.release()
    psum.release()


# ---------------------------------------------------------------------------
# host-side input prep (numpy; all weight arrays already in SBUF layout)
# ---------------------------------------------------------------------------

BF = ml_dtypes.bfloat16


def _prep_shared(w1, b1, w2, b2, w3, b3, w4, b4, w5, b5):
    f = np.float32
    # conv1: rows r = dyo*33 + dx*3 + c, row 99 = bias(ones); 4 passes dy=3p+dyo
    lw1 = np.zeros((100, 4 * 64), f)
    for p in range(4):
        for dyo in range(3):
            dy = 3 * p + dyo
            if dy > 10:
                continue
            for dx in range(11):
                for c in range(3):
                    lw1[dyo * 33 + dx * 3 + c, p * 64:(p + 1) * 64] = w1[:, c, dy, dx]
    lw1[99, 0:64] = b1
    # conv2: [128, (dy,j,m)]: j<3 -> rows s*64+c = ch c at dx=2j+s; j=3 -> dx=6
    lw2 = np.zeros((128, 7 * 4 * 192), f)
    for dy in range(7):
        for j in range(3):
            for s in range(2):
                lw2[s * 64:(s + 1) * 64, (dy * 4 + j) * 192:(dy * 4 + j + 1) * 192] = \
                    w2[:, :, dy, 2 * j + s].T
        lw2[0:64, (dy * 4 + 3) * 192:(dy * 4 + 4) * 192] = w2[:, :, dy, 6].T
    # col 0: mc0 bias; col 2: mc1 bias duplicated onto both partition halves
    # (the col-tiled mc1 pair holds two output rows of the same 64 channels)
    lb2 = np.zeros((128, 3), f)
    lb2[:, 0] = b2[0:128]
    lb2[0:64, 1] = b2[128:192]
    lb2[0:64, 2] = b2[128:192]
    lb2[64:128, 2] = b2[128:192]
    # conv3: [128, 15360]: cols blk*384+m (kc0, K=128); kc1 packs dx pairs
    # (0,1),(2,3),(4,) per dy at 9600+(dy*3+j)*384 — rows 0:64 = dx=2j,
    # rows 64:128 = dx=2j+1 (X3b partitions 64:128 hold x+1-shifted acts)
    lw3 = np.zeros((128, 15360), f)
    for dy in range(5):
        for dx in range(5):
            blk = dy * 5 + dx
            lw3[:, blk * 384:(blk + 1) * 384] = w3[:, 0:128, dy, dx].T
        for j in range(3):
            co = 9600 + (dy * 3 + j) * 384
            lw3[0:64, co:co + 384] = w3[:, 128:192, dy, 2 * j].T
            if j < 2:
                lw3[64:128, co:co + 384] = w3[:, 128:192, dy, 2 * j + 1].T
    lb3 = np.zeros((128, 3), f)
    lb3[:, 0] = b3[0:128]; lb3[:, 1] = b3[128:256]; lb3[:, 2] = b3[256:384]
    # conv4 / conv5: [128, (o, m)] with o = (dy*3+dx)*nkc + kc
    lw4 = np.zeros((128, 27 * 256), f)
    for dy in range(3):
        for dx in range(3):
            for kc in range(3):
                o = (dy * 3 + dx) * 3 + kc
                lw4[:, o * 256:(o + 1) * 256] = w4[:, kc * 128:(kc + 1) * 128, dy, dx].T
    lb4 = np.stack([b4[0:128], b4[128:256]], axis=1).astype(f)
    lw5 = np.zeros((128, 18 * 256), f)
    for dy in range(3):
        for dx in range(3):
            for kc in range(2):
                o = (dy * 3 + dx) * 2 + kc
                lw5[:, o * 256:(o + 1) * 256] = w5[:, kc * 128:(kc + 1) * 128, dy, dx].T
    lb5 = np.stack([b5[0:128], b5[128:256]], axis=1).astype(f)
    return dict(lw1=np.ascontiguousarray(lw1.astype(BF)),
                lw2=np.ascontiguousarray(lw2.astype(BF)), lb2=lb2,
                lw3=np.ascontiguousarray(lw3.astype(BF)), lb3=lb3,
                lw4=np.ascontiguousarray(lw4.astype(BF)), lb4=lb4,
                lw5=np.ascontiguousarray(lw5.astype(BF)), lb5=lb5)


def _prep_x13(x):
    """x [B,3,32,32] -> per-core [16, 100, 2*41*32] im2col-packed bf16."""
    f = np.float32
    xpad = np.zeros((B, 3, 44, 42), f)
    xpad[:, :, 5:37, 5:37] = x
    X = np.zeros((100, B, 41, 32), f)
    for dyo in range(3):
        for dx in range(11):
            for c in range(3):
                X[dyo * 33 + dx * 3 + c] = xpad[:, c, dyo:dyo + 41, dx:dx + 32]
    X[99] = 1.0
    out = []
    for r in range(N_CORES):
        pc = X[:, r * BC:(r + 1) * BC]  # [100, 32, 41, 32]
        pc = pc.reshape(100, 16, 2 * 41 * 32).transpose(1, 0, 2)
        out.append(np.ascontiguousarray(pc.astype(BF)))
    return out


def _prep_fc(fw1, fb1, fw2, fb2, fw3, fb3):
    f = np.float32
    outs = []
    for r in range(N_CORES):
        sl = slice(512 * r, 512 * (r + 1))
        # fw1s [128, (yx, cc, m)]: fw1[512r+m, (cc*128+k)*16+yx]
        fw1s = fw1[sl].reshape(512, 2, 128, 16).transpose(2, 3, 1, 0).reshape(128, -1)
        fb1s = fb1[sl].reshape(4, 128).T
        # fw2s [128, (kc, m)]: fw2[512r+m, kc*128+k]
        fw2s = fw2[sl].reshape(512, 32, 128).transpose(2, 1, 0).reshape(128, -1)
        fb2s = fb2[sl].reshape(4, 128).T
        # fw3s [128, (kc, m)]: fw3[m, 512r + kc*128 + k]
        fw3s = fw3[:, sl].reshape(100, 4, 128).transpose(2, 1, 0).reshape(128, -1)
        fb3s = (fb3 / N_CORES).reshape(100, 1)
        outs.append(dict(
            fw1s=np.ascontiguousarray(fw1s.astype(BF)),
            fb1s=np.ascontiguousarray(fb1s.astype(f)),
            fw2s=np.ascontiguousarray(fw2s.astype(BF)),
            fb2s=np.ascontiguousarray(fb2s.astype(f)),
            fw3s=np.ascontiguousarray(fw3s.astype(BF)),
            fb3s=np.ascontiguousarray(fb3s.astype(f)),
        ))
    return outs


_CACHE = {}

_SHAPES = dict(
    x13=(16, 100, 2 * 41 * 32), lw1=(100, 4 * 64),
    lw2=(128, 7 * 4 * 192), lb2=(128, 3),
    lw3=(128, 15360), lb3=(128, 3),
    lw4=(128, 27 * 256), lb4=(128, 2),
    lw5=(128, 18 * 256), lb5=(128, 2),
    fw1s=(128, 32 * 512), fb1s=(128, 4),
    fw2s=(128, 32 * 512), fb2s=(128, 4),
    fw3s=(128, 4 * 100), fb3s=(100, 1),
)

_BF16_INPUTS = {"x13", "lw1", "lw2", "lw3", "lw4", "lw5", "fw1s", "fw2s", "fw3s"}


def _build():
    if "nc" in _CACHE:
        return _CACHE["nc"]
    nc = bacc.Bacc("TRN2", target_bir_lowering=False, debug=False,
                   num_devices=N_CORES)

    def _dt(name):
        return BF16 if name in _BF16_INPUTS else F32
    t = {name: nc.dram_tensor(
            name, list(shape), _dt(name), kind="ExternalInput").ap()
         for name, shape in _SHAPES.items()}
    if DEBUG_DUMPS:
        for name, shape in _DBG_SHAPES.items():
            t[name] = nc.dram_tensor(
                name, list(shape), BF16, kind="ExternalOutput").ap()
    yout = nc.dram_tensor("yout", [100, B], BF16, kind="ExternalOutput").ap()
    with tile.TileContext(nc) as tc:
        _emit(nc, tc, t, yout)
    nc.compile()
    _CACHE["nc"] = nc
    return nc


def kernel(x, w1, b1, w2, b2, w3, b3, w4, b4, w5, b5,
           fw1, fb1, fw2, fb2, fw3, fb3):
    args = [np.asarray(a, np.float32) for a in
            (x, w1, b1, w2, b2, w3, b3, w4, b4, w5, b5, fw1, fb1, fw2, fb2, fw3, fb3)]
    (x, w1, b1, w2, b2, w3, b3, w4, b4, w5, b5,
     fw1, fb1, fw2, fb2, fw3, fb3) = args
    nc = _build()
    shared = _prep_shared(w1, b1, w2, b2, w3, b3, w4, b4, w5, b5)
    x13s = _prep_x13(x)
    fcs = _prep_fc(fw1, fb1, fw2, fb2, fw3, fb3)
    in_maps = [{**shared, "x13": x13s[r], **fcs[r]} for r in range(N_CORES)]
    res = run_bass_kernel_spmd(nc, in_maps, list(range(N_CORES)))
    _CACHE["last_results"] = res.results
    y = res.results[0]["yout"]  # [100, 256] bf16
    return np.ascontiguousarray(y.T.astype(np.float32))


# revision 37
# speedup vs baseline: 1.4325x; 1.0188x over previous
"""AlexNet-style CNN forward pass on 8 Trainium2 NeuronCores.

Strategy (v1):
  - Convs data-parallel: batch 256 -> 32 per core, channels on partitions,
    conv = sum of shifted matmuls over kernel offsets (weights replicated).
  - All conv weights/activations in bf16 (PSUM accumulates fp32); halves
    DMA + SBUF traffic and enables FWL weight loads.
  - Activations laid out batch-innermost ([ch, y, x, b]) so every matmul
    rhs is runs of 32 contiguous elements (the fp32 baseline's 8-elem
    strided runs slowed the PE rhs stream ~1.3x).
  - conv1 (cin=3) uses host-packed im2col rows (3 dy-offsets x 11 dx x 3 ch
    + ones row for fused bias -> K=100), 8 input chunks DMA'd on the sync
    queue (weights go on the scalar queue) so the first matmul starts ~8us
    in instead of 53us.
  - conv2 uses an x-shifted duplicate of its input (K=128 = 2 dx-offsets
    x 64 ch); conv3 likewise packs two dx offsets via a shifted copy.
  - FC layers model-parallel: each core owns 512 rows of fc1/fc2 and 512
    K-columns of fc3.  pool5 is AllGathered in 2 chunks that overlap fc1's
    K-accumulation into persistent PSUM; fc1 output AllGathered, fc3
    partials AllReduced.
"""

import os

import ml_dtypes
import numpy as np

import concourse.bass as bass
import concourse.mybir as mybir
import concourse.tile as tile
from concourse import bacc
from concourse.bass_utils import run_bass_kernel_spmd

N_CORES = 8
B = 256
BC = B // N_CORES  # 32 images per core

F32 = mybir.dt.float32
BF16 = mybir.dt.bfloat16
RELU = mybir.ActivationFunctionType.Relu
IDENT = mybir.ActivationFunctionType.Identity

DEBUG_DUMPS = os.environ.get("BASSDBG") == "1"

_DBG_SHAPES = dict(
    dX2s=(128, 22 * 23 * 32), dX3a=(128, 12 * 12 * 32), dX3b=(128, 12 * 12 * 32),
    dX4_0=(128, 3200), dX4_1=(128, 3200), dX4_2=(128, 3200),
    dX5_0=(128, 3200), dX5_1=(128, 3200),
    dP5_0=(128, 512), dP5_1=(128, 512),
    dF1=(128, 1024), dF2=(128, 1024),
)


def _emit(nc, tc, t, yout):
    """Emit the whole network. t: dict name -> DRAM AP."""
    sync = nc.sync
    act = nc.scalar
    dve = nc.vector
    pool_e = nc.gpsimd

    psum = tc.alloc_tile_pool(name="psum", bufs=6, space="PSUM")
    scr = tc.alloc_tile_pool(name="scr", bufs=1, side="left")
    dram = tc.alloc_tile_pool(name="dram", bufs=1, space="DRAM")

    # ---------------- phase pools ----------------
    p_w12 = tc.alloc_tile_pool(name="p_w12", bufs=1, side="left")
    p_x2s = tc.alloc_tile_pool(name="p_x2s", bufs=1, side="left")
    p_x13 = tc.alloc_tile_pool(name="p_x13", bufs=4, side="left")

    # x13 streams as 16 chunks of 2 images alternating between the two
    # HWDGE queues (sync + scalar); each queue sustains only ~100 GB/s.
    # conv1/conv2 weights interleave on the scalar queue.
    lw1 = p_w12.tile([100, 4 * 64], BF16)
    act.dma_start(lw1[:], t["lw1"][:])
    xts = []
    for ch in range(16):
        xt = p_x13.tile([100, 2 * 41 * 32], BF16, tag="x13")
        (sync if ch % 2 == 0 else act).dma_start(xt[:], t["x13"][ch])
        xts.append(xt)
        if ch == 5:
            lw2 = p_w12.tile([128, 7 * 4 * 192], BF16)
            act.dma_start(lw2[:], t["lw2"][:])
        if ch == 7:
            lb2 = p_w12.tile([128, 3], F32)
            act.dma_start(lb2[:], t["lb2"][:])

    # conv2 input: [128, 22y, 23x, 32b]; rows 0:64 ch c at x, rows 64:128
    # ch c at x+1 (b innermost so matmul rhs reads runs of 32 elems)
    X2s = p_x2s.tile([128, 22 * 23 * 32], BF16)
    pool_e.memset(X2s[:].bitcast(F32), 0.0)
    X2sv = X2s.rearrange("p (y x b) -> p y x b", y=22, x=23, b=32)

    # ---------------- conv1 + pool1 ----------------
    _sid = nc.enter_named_scope("L1_conv1", False)[0]
    for ch in range(16):
        xtv = xts[ch].rearrange("k (b y x) -> k b y x", b=2, y=41, x=32)
        for bl in range(2):
            b = ch * 2 + bl
            for h in range(2):  # vertical half of the 32x32 output
                ps = psum.tile([64, 512], F32, tag="ps")
                for pi in range(4):
                    p = 3 * pi
                    nc.tensor.matmul(
                        ps[:],
                        lw1[:, pi * 64:(pi + 1) * 64],
                        xtv[:, bl, h * 16 + p:h * 16 + p + 16, :],
                        start=(pi == 0), stop=(pi == 3),
                    )
                # evict+relu (bias came in via the ones-row), then 2x2 maxpool
                s1 = scr.tile([64, 512], BF16, tag="ev1", bufs=3)
                act.activation(s1[:], ps[:], RELU)
                sv = s1.rearrange("m (y x) -> m y x", y=16, x=32)
                m = scr.tile([64, 256], BF16, tag="m", bufs=4)
                mv = m.rearrange("m (y x) -> m y x", y=16, x=16)
                dve.tensor_max(mv, sv[:, :, 0::2], sv[:, :, 1::2])
                y0 = h * 8 + 3
                dve.tensor_max(
                    X2sv[0:64, y0:y0 + 8, 3:19, b],
                    mv[:, 0::2, :], mv[:, 1::2, :])
    # duplicate into the x+1-shifted partition block (b-inner: shift by 32
    # elems).  Flat-shifted copy in 3 slices on 3 DMA paths so conv2's
    # first rows can start early; wrapped elements land in dead pad columns.
    TOT = 22 * 23 * 32
    cuts = [0, TOT // 2, TOT - 32]
    for si, eng in enumerate((sync, act)):
        lo, hi = cuts[si], cuts[si + 1]
        eng.dma_start(X2s[64:128, lo:hi], X2s[0:64, lo + 32:hi + 32])
    p_x13.release()
    nc.leave_named_scope("L1_conv1", _sid, False)

    # conv3 weights (prefetch during conv2) + conv3 input buffers
    p_w3 = tc.alloc_tile_pool(name="p_w3", bufs=1, side="right")
    p_x3 = tc.alloc_tile_pool(name="p_x3", bufs=1, side="right")
    lw3 = p_w3.tile([128, 15360], BF16)
    act.dma_start(lw3[:], t["lw3"][:])
    lb3 = p_w3.tile([128, 3], F32)
    act.dma_start(lb3[:], t["lb3"][:])
    # X3a: [128ch, 12y, 12x, 32b] (pad 2); X3b rows 0:64 = ch 128:192,
    # rows 64:128 its x+1-shifted copy (conv3 kc1 packs two dx per K=128)
    X3a = p_x3.tile([128, 12 * 12 * 32], BF16)
    X3b = p_x3.tile([128, 12 * 12 * 32], BF16)
    pool_e.memset(X3a[:].bitcast(F32), 0.0)
    pool_e.memset(X3b[:].bitcast(F32), 0.0)
    X3av = X3a.rearrange("p (y x b) -> p y x b", y=12, x=12, b=32)
    X3bv = X3b.rearrange("p (y x b) -> p y x b", y=12, x=12, b=32)

    # ---------------- conv2 + pool2 ----------------
    _sid = nc.enter_named_scope("L2_conv2", False)[0]
    lw2v = lw2.rearrange("k (a j m) -> k a j m", a=7, j=4, m=192)
    DJ = [(dy, j) for dy in range(7) for j in range(4)]

    def c2_rhs(y, dy, j):
        K = 128 if j < 3 else 64
        xoff = 2 * j if j < 3 else 6
        return X2sv[0:K, y + dy, xoff:xoff + 16, :]

    for yp in range(8):  # output row pairs
        ye, yo = 2 * yp, 2 * yp + 1
        # mc0 (M=128): one full-array psum tile per row
        s2keep = None
        for y in (ye, yo):
            ps = psum.tile([128, 512], F32, tag="ps")
            for i, (dy, j) in enumerate(DJ):
                K = 128 if j < 3 else 64
                nc.tensor.matmul(
                    ps[:], lw2v[0:K, dy, j, 0:128], c2_rhs(y, dy, j),
                    start=(i == 0), stop=(i == 27),
                )
            s2 = scr.tile([128, 512], BF16, tag="ev", bufs=4)
            act.activation(s2[:], ps[:], RELU, bias=lb2[:, 0:1])
            if y == ye:
                s2keep = s2
            else:
                pm = scr.tile([128, 512], BF16, tag="pm", bufs=2)
                dve.tensor_max(pm[:], s2keep[:], s2[:])
                pmv = pm.rearrange("m (x b) -> m x b", x=16, b=32)
                dve.tensor_max(X3av[0:128, yp + 2, 2:10, :],
                               pmv[:, 0::2, :], pmv[:, 1::2, :])
        # mc1 (M=64)
        s2keep = None
        for y in (ye, yo):
            ps = psum.tile([64, 512], F32, tag="ps")
            for i, (dy, j) in enumerate(DJ):
                K = 128 if j < 3 else 64
                nc.tensor.matmul(
                    ps[:], lw2v[0:K, dy, j, 128:192], c2_rhs(y, dy, j),
                    start=(i == 0), stop=(i == 27),
                )
            s2 = scr.tile([64, 512], BF16, tag="evb", bufs=4)
            act.activation(s2[:], ps[:], RELU, bias=lb2[0:64, 1:2])
            if y == ye:
                s2keep = s2
            else:
                pm = scr.tile([64, 512], BF16, tag="pmb", bufs=2)
                dve.tensor_max(pm[:], s2keep[:], s2[:])
                pmv = pm.rearrange("m (x b) -> m x b", x=16, b=32)
                dve.tensor_max(X3bv[0:64, yp + 2, 2:10, :],
                               pmv[:, 0::2, :], pmv[:, 1::2, :])
    # x+1-shifted duplicate for conv3's dx pairing (flat +32 shift)
    X3TOT = 12 * 12 * 32
    sync.dma_start(X3b[64:128, 0:X3TOT // 2], X3b[0:64, 32:X3TOT // 2 + 32])
    act.dma_start(X3b[64:128, X3TOT // 2:X3TOT - 32], X3b[0:64, X3TOT // 2 + 32:X3TOT])
    if DEBUG_DUMPS:
        sync.dma_start(t["dX2s"][:], X2s[:])
        sync.dma_start(t["dX3a"][:], X3a[:])
        sync.dma_start(t["dX3b"][:], X3b[:])
    nc.leave_named_scope("L2_conv2", _sid, False)
    p_x2s.release()
    p_w12.release()

    # conv4/5 weights (prefetch during conv3) + conv4 input buffers
    p_w45 = tc.alloc_tile_pool(name="p_w45", bufs=1, side="left")
    p_x4 = tc.alloc_tile_pool(name="p_x4", bufs=1, side="left")
    lw4 = p_w45.tile([128, 27 * 256], BF16)
    act.dma_start(lw4[:], t["lw4"][:])
    lb4 = p_w45.tile([128, 2], F32)
    act.dma_start(lb4[:], t["lb4"][:])
    lw5 = p_w45.tile([128, 18 * 256], BF16)
    act.dma_start(lw5[:], t["lw5"][:])
    lb5 = p_w45.tile([128, 2], F32)
    act.dma_start(lb5[:], t["lb5"][:])
    X4 = []
    X4v = []
    for i in range(3):
        X4.append(p_x4.tile([128, 10 * 10 * 32], BF16, name=f"X4_{i}"))
        pool_e.memset(X4[i][:].bitcast(F32), 0.0)
        X4v.append(X4[i].rearrange("p (y x b) -> p y x b", y=10, x=10, b=32))

    _sid = nc.enter_named_scope("L3_conv3", False)[0]
    # ---------------- conv3 ----------------
    for tt in range(4):  # output row pairs; psum free = 2y * 8x * 32b
        for mc in range(3):
            ps = psum.tile([128, 512], F32, tag="ps")
            first = True
            for dy in range(5):
                for dx in range(5):
                    blk = dy * 5 + dx
                    nc.tensor.matmul(
                        ps[:],
                        lw3[0:128, blk * 384 + mc * 128:blk * 384 + mc * 128 + 128],
                        X3av[0:128, 2 * tt + dy:2 * tt + dy + 2, dx:dx + 8, :],
                        start=first, stop=False,
                    )
                    first = False
                for j in range(3):  # kc1: dx pairs (0,1),(2,3),(4,)
                    K = 128 if j < 2 else 64
                    co = 9600 + (dy * 3 + j) * 384
                    nc.tensor.matmul(
                        ps[:],
                        lw3[0:K, co + mc * 128:co + mc * 128 + 128],
                        X3bv[0:K, 2 * tt + dy:2 * tt + dy + 2, 2 * j:2 * j + 8, :],
                        start=False, stop=(dy == 4 and j == 2),
                    )
            act.activation(
                X4v[mc][:, 2 * tt + 1:2 * tt + 3, 1:9, :],
                ps.rearrange("m (y x b) -> m y x b", y=2, x=8, b=32),
                RELU, bias=lb3[:, mc:mc + 1])
    if DEBUG_DUMPS:
        for i in range(3):
            sync.dma_start(t[f"dX4_{i}"][:], X4[i][:])
    nc.leave_named_scope("L3_conv3", _sid, False)
    p_x3.release()
    p_w3.release()

    # fc1 weights (prefetch during conv4) + conv5 input buffers
    p_fw1 = tc.alloc_tile_pool(name="p_fw1", bufs=1, side="right")
    p_x5 = tc.alloc_tile_pool(name="p_x5", bufs=1, side="right")
    fw1 = p_fw1.tile([128, 32 * 512], BF16)
    act.dma_start(fw1[:], t["fw1s"][:])
    fb1 = p_fw1.tile([128, 4], F32)
    act.dma_start(fb1[:], t["fb1s"][:])
    X5 = []
    X5v = []
    for i in range(2):
        X5.append(p_x5.tile([128, 10 * 10 * 32], BF16, name=f"X5_{i}"))
        pool_e.memset(X5[i][:].bitcast(F32), 0.0)
        X5v.append(X5[i].rearrange("p (y x b) -> p y x b", y=10, x=10, b=32))

    _sid = nc.enter_named_scope("L4_conv4", False)[0]
    # ---------------- conv4 ----------------
    lw4v = lw4.rearrange("k (o m) -> k o m", o=27)
    for tt in range(4):
        for mc in range(2):
            ps = psum.tile([128, 512], F32, tag="ps")
            first = True
            for dy in range(3):
                for dx in range(3):
                    for kc in range(3):
                        o = (dy * 3 + dx) * 3 + kc
                        nc.tensor.matmul(
                            ps[:],
                            lw4v[:, o, mc * 128:mc * 128 + 128],
                            X4v[kc][:, 2 * tt + dy:2 * tt + dy + 2, dx:dx + 8, :],
                            start=first, stop=(o == 26),
                        )
                        first = False
            act.activation(
                X5v[mc][:, 2 * tt + 1:2 * tt + 3, 1:9, :],
                ps.rearrange("m (y x b) -> m y x b", y=2, x=8, b=32),
                RELU, bias=lb4[:, mc:mc + 1])
    if DEBUG_DUMPS:
        for i in range(2):
            sync.dma_start(t[f"dX5_{i}"][:], X5[i][:])
    nc.leave_named_scope("L4_conv4", _sid, False)
    p_x4.release()

    # fc2/fc3 weights (DMA overlaps conv5 + the gathers + fc1)
    p_fw2 = tc.alloc_tile_pool(name="p_fw2", bufs=1, side="left")
    fw2 = p_fw2.tile([128, 32 * 512], BF16)
    act.dma_start(fw2[:], t["fw2s"][:])
    fb2 = p_fw2.tile([128, 4], F32)
    act.dma_start(fb2[:], t["fb2s"][:])
    fw3 = p_fw2.tile([128, 4 * 100], BF16)
    act.dma_start(fw3[:], t["fw3s"][:])
    fb3 = p_fw2.tile([100, 1], F32)
    act.dma_start(fb3[:], t["fb3s"][:])

    # pool5 output: [128ch, 4t, 4x, 32b] per channel half
    p_p5 = tc.alloc_tile_pool(name="p_p5", bufs=1, side="left")
    P5 = [p_p5.tile([128, BC * 16], BF16, name=f"P5_{i}") for i in range(2)]

    # fc1 persistent psum: 4 mc-quarter outputs of [128, 256] in 2 banks
    p_psF = tc.alloc_tile_pool(name="psumF", bufs=1, space="PSUM")
    psF = p_psF.tile([128, 1024], F32)

    # pool5 gather chunks: 4 x (channel half, tt-pair); each 64KB stage ->
    # 512KB gather, launched as soon as its rows complete so gather latency
    # hides under the remaining conv5 compute + earlier fc1 chunks
    cin5 = [dram.tile([128, 256], BF16, name=f"cin5_{i}") for i in range(4)]
    g1 = [dram.tile([N_CORES, 128, 256], BF16, addr_space="Shared",
                    name=f"g1c{i}") for i in range(4)]

    # tiny warm-up collective: the first real gather after a long CC-idle
    # stretch pays ~11us of trigger latency; this keeps the CC path warm
    cw_in = dram.tile([128, 8], BF16)
    cw_out = dram.tile([N_CORES, 128, 8], BF16, addr_space="Shared")

    # fc1 state (chunks interleave with conv5's emission)
    p_f1 = tc.alloc_tile_pool(name="p_f1", bufs=1, side="left")
    F1 = p_f1.tile([128, 4 * B], BF16)
    p_h1 = tc.alloc_tile_pool(name="p_h1", bufs=2, side="left")
    fw1v = fw1.rearrange("k (y c m) -> k y c m", y=16, c=2, m=512)

    def fc1_chunk(j):
        cc, th = j // 2, j % 2
        H1 = p_h1.tile([128, 8 * 256], BF16, tag="h1")
        H1v = H1.rearrange("c (r t x b) -> c r t x b", r=8, t=2, x=4, b=32)
        H1f = H1.rearrange("c (r f) -> c r f", r=8)
        for rh in range(2):  # split the load across both HWDGE queues
            (sync if rh == 0 else act).dma_start(
                H1f[:, rh * 4:(rh + 1) * 4],
                g1[j][rh * 4:(rh + 1) * 4].rearrange("r c f -> c r f"))
        for tti in range(2):
            tt = 2 * th + tti
            for x in range(4):
                yx = tt * 4 + x
                for mq in range(4):
                    # start=True clears has_written for the WHOLE bank;
                    # mq pairs (0,1) and (2,3) share a bank, so only the
                    # bank's first matmul may set it — the partner region
                    # inits via overwrite-where-unset semantics.
                    nc.tensor.matmul(
                        psF[:, mq * 256:(mq + 1) * 256],
                        fw1v[:, yx, cc, mq * 128:mq * 128 + 128],
                        H1v[:, :, tti, x, :],
                        start=(j == 0 and tti == 0 and x == 0 and mq % 2 == 0),
                        stop=(j == 3 and tti == 1 and x == 3),
                        skip_group_check=True,
                    )

    _sid = nc.enter_named_scope("L5_conv5", False)[0]
    # ---------------- conv5 + pool5 (mc-outer) + interleaved fc1 ------------
    lw5v = lw5.rearrange("k (o m) -> k o m", o=18)
    for mc in range(2):
        for tt in range(4):
            ps = psum.tile([128, 512], F32, tag="ps")
            first = True
            for dy in range(3):
                for dx in range(3):
                    for kc in range(2):
                        o = (dy * 3 + dx) * 2 + kc
                        nc.tensor.matmul(
                            ps[:],
                            lw5v[:, o, mc * 128:mc * 128 + 128],
                            X5v[kc][:, 2 * tt + dy:2 * tt + dy + 2, dx:dx + 8, :],
                            start=first, stop=(o == 17),
                        )
                        first = False
            s5 = scr.tile([128, 512], BF16, tag="ev", bufs=4)
            act.activation(s5[:], ps[:], RELU, bias=lb5[:, mc:mc + 1])
            s5v = s5.rearrange("m (y x b) -> m y x b", y=2, x=8, b=32)
            pm = scr.tile([128, 256], BF16, tag="pm5", bufs=2)
            dve.tensor_max(pm[:], s5v[:, 0, :, :], s5v[:, 1, :, :])
            pmv = pm.rearrange("m (x b) -> m x b", x=8, b=32)
            p5v = P5[mc].rearrange("p (t x b) -> p t x b", t=4, x=4, b=32)
            dve.tensor_max(p5v[:, tt, :, :], pmv[:, 0::2, :], pmv[:, 1::2, :])
            if mc == 0 and tt == 0:
                sync.dma_start(cw_in[:], lw5[:, 0:8])
                pool_e.collective_compute(
                    "AllGather", mybir.AluOpType.bypass,
                    replica_groups=[list(range(N_CORES))],
                    ins=[cw_in.opt()], outs=[cw_out.opt()])
            if tt % 2 == 1:
                j = mc * 2 + tt // 2
                th = tt // 2
                sync.dma_start(cin5[j][:],
                               P5[mc][:, th * 256:(th + 1) * 256])
                pool_e.collective_compute(
                    "AllGather", mybir.AluOpType.bypass,
                    replica_groups=[list(range(N_CORES))],
                    ins=[cin5[j].opt()], outs=[g1[j].opt()])
                if j >= 1:
                    fc1_chunk(j - 1)
    if DEBUG_DUMPS:
        for i in range(2):
            sync.dma_start(t[f"dP5_{i}"][:], P5[i][:])
    nc.leave_named_scope("L5_conv5", _sid, False)
    p_x5.release()

    _sid = nc.enter_named_scope("F1_fc1", False)[0]
    # ---------------- fc1: last gathered chunk + eviction -------------------
    fc1_chunk(3)
    # evict + stage + gather fc1 output in two halves so the second gather
    # overlaps fc2's first-half accumulation
    cin6 = [dram.tile([128, 2 * B], BF16, name=f"cin6_{h}") for h in range(2)]
    g2h = [dram.tile([N_CORES, 128, 2 * B], BF16, addr_space="Shared",
                     name=f"g2_{h}") for h in range(2)]
    for half in range(2):
        for mq in (2 * half, 2 * half + 1):
            act.activation(F1[:, mq * B:(mq + 1) * B],
                           psF[:, mq * 256:(mq + 1) * 256],
                           RELU, bias=fb1[:, mq:mq + 1])
        sync.dma_start(cin6[half][:], F1[:, half * 2 * B:(half + 1) * 2 * B])
        pool_e.collective_compute(
            "AllGather", mybir.AluOpType.bypass,
            replica_groups=[list(range(N_CORES))],
            ins=[cin6[half].opt()], outs=[g2h[half].opt()])
    if DEBUG_DUMPS:
        sync.dma_start(t["dF1"][:], F1[:])
    p_h1.release()
    p_fw1.release()
    p_psF.release()

    nc.leave_named_scope("F1_fc1", _sid, False)
    _sid = nc.enter_named_scope("G2_gather", False)[0]
    # ---------------- load gathered fc1 ----------------
    p_h2 = tc.alloc_tile_pool(name="p_h2", bufs=1, side="right")
    H2 = p_h2.tile([128, N_CORES * 4 * B], BF16)
    H2v = H2.rearrange("c (r h f) -> c r h f", r=N_CORES, h=2)
    for half in range(2):
        for rh in range(2):  # split each 1MB load across both HWDGE queues
            (sync if rh == 0 else act).dma_start(
                H2v[:, rh * 4:(rh + 1) * 4, half],
                g2h[half][rh * 4:(rh + 1) * 4].rearrange("r c f -> c r f"))
    p_f1.release()

    nc.leave_named_scope("G2_gather", _sid, False)
    _sid = nc.enter_named_scope("F2_fc2", False)[0]
    # ---------------- fc2 (half-0 kc blocks first: they arrive first) ------
    p_f2 = tc.alloc_tile_pool(name="p_f2", bufs=1, side="left")
    F2 = p_f2.tile([128, 4 * B], BF16)
    fw2v = fw2.rearrange("k (a m) -> k a m", a=32)
    kcs = [r * 4 + 2 * half + q
           for half in range(2) for r in range(8) for q in range(2)]
    for mc in range(4):
        ps = psum.tile([128, B], F32, tag="ps")
        for i, kc in enumerate(kcs):
            nc.tensor.matmul(
                ps[:], fw2v[:, kc, mc * 128:mc * 128 + 128],
                H2[:, kc * B:(kc + 1) * B],
                start=(i == 0), stop=(i == 31))
        act.activation(F2[:, mc * B:(mc + 1) * B], ps[:], RELU, bias=fb2[:, mc:mc + 1])
    if DEBUG_DUMPS:
        sync.dma_start(t["dF2"][:], F2[:])
    p_h2.release()

    nc.leave_named_scope("F2_fc2", _sid, False)
    _sid = nc.enter_named_scope("F3_fc3", False)[0]
    # ---------------- fc3 (partial over this core's 512 K) + AllReduce ----
    fw3v = fw3.rearrange("k (a m) -> k a m", a=4)
    ps = psum.tile([128, B], F32, tag="ps")
    for kc in range(4):
        nc.tensor.matmul(
            ps[0:100, :], fw3v[:, kc, :], F2[:, kc * B:(kc + 1) * B],
            start=(kc == 0), stop=(kc == 3))
    s3 = scr.tile([128, 512], BF16, tag="ev3", bufs=1)
    act.activation(s3[0:100, 0:B], ps[0:100, :], IDENT, bias=fb3[:])  # + fb3/8
    cin7 = dram.tile([100, B], BF16)
    sync.dma_start(cin7[:], s3[0:100, 0:B])
    g3 = dram.tile([100, B], BF16)
    pool_e.collective_compute(
        "AllReduce", mybir.AluOpType.add,
        replica_groups=[list(range(N_CORES))],
        ins=[cin7.opt()], outs=[g3.opt()])
    sync.dma_start(yout[:], g3[:])
    nc.leave_named_scope("F3_fc3", _sid, False)
    p_f2.release()
    p_p5.release()
    p_fw2.release()
    p_w45.release()

    scr.release()
    dram.release()
    psum.release()


# ---------------------------------------------------------------------------
# host-side input prep (numpy; all weight arrays already in SBUF layout)
# ---------------------------------------------------------------------------

BF = ml_dtypes.bfloat16


def _prep_shared(w1, b1, w2, b2, w3, b3, w4, b4, w5, b5):
    f = np.float32
    # conv1: rows r = dyo*33 + dx*3 + c, row 99 = bias(ones); 4 passes dy=3p+dyo
    lw1 = np.zeros((100, 4 * 64), f)
    for p in range(4):
        for dyo in range(3):
            dy = 3 * p + dyo
            if dy > 10:
                continue
            for dx in range(11):
                for c in range(3):
                    lw1[dyo * 33 + dx * 3 + c, p * 64:(p + 1) * 64] = w1[:, c, dy, dx]
    lw1[99, 0:64] = b1
    # conv2: [128, (dy,j,m)]: j<3 -> rows s*64+c = ch c at dx=2j+s; j=3 -> dx=6
    lw2 = np.zeros((128, 7 * 4 * 192), f)
    for dy in range(7):
        for j in range(3):
            for s in range(2):
                lw2[s * 64:(s + 1) * 64, (dy * 4 + j) * 192:(dy * 4 + j + 1) * 192] = \
                    w2[:, :, dy, 2 * j + s].T
        lw2[0:64, (dy * 4 + 3) * 192:(dy * 4 + 4) * 192] = w2[:, :, dy, 6].T
    # col 0: mc0 bias; col 2: mc1 bias duplicated onto both partition halves
    # (the col-tiled mc1 pair holds two output rows of the same 64 channels)
    lb2 = np.zeros((128, 3), f)
    lb2[:, 0] = b2[0:128]
    lb2[0:64, 1] = b2[128:192]
    lb2[0:64, 2] = b2[128:192]
    lb2[64:128, 2] = b2[128:192]
    # conv3: [128, 15360]: cols blk*384+m (kc0, K=128); kc1 packs dx pairs
    # (0,1),(2,3),(4,) per dy at 9600+(dy*3+j)*384 — rows 0:64 = dx=2j,
    # rows 64:128 = dx=2j+1 (X3b partitions 64:128 hold x+1-shifted acts)
    lw3 = np.zeros((128, 15360), f)
    for dy in range(5):
        for dx in range(5):
            blk = dy * 5 + dx
            lw3[:, blk * 384:(blk + 1) * 384] = w3[:, 0:128, dy, dx].T
        for j in range(3):
            co = 9600 + (dy * 3 + j) * 384
            lw3[0:64, co:co + 384] = w3[:, 128:192, dy, 2 * j].T
            if j < 2:
                lw3[64:128, co:co + 384] = w3[:, 128:192, dy, 2 * j + 1].T
    lb3 = np.zeros((128, 3), f)
    lb3[:, 0] = b3[0:128]; lb3[:, 1] = b3[128:256]; lb3[:, 2] = b3[256:384]
    # conv4 / conv5: [128, (o, m)] with o = (dy*3+dx)*nkc + kc
    lw4 = np.zeros((128, 27 * 256), f)
    for dy in range(3):
        for dx in range(3):
            for kc in range(3):
                o = (dy * 3 + dx) * 3 + kc
                lw4[:, o * 256:(o + 1) * 256] = w4[:, kc * 128:(kc + 1) * 128, dy, dx].T
    lb4 = np.stack([b4[0:128], b4[128:256]], axis=1).astype(f)
    lw5 = np.zeros((128, 18 * 256), f)
    for dy in range(3):
        for dx in range(3):
            for kc in range(2):
                o = (dy * 3 + dx) * 2 + kc
                lw5[:, o * 256:(o + 1) * 256] = w5[:, kc * 128:(kc + 1) * 128, dy, dx].T
    lb5 = np.stack([b5[0:128], b5[128:256]], axis=1).astype(f)
    return dict(lw1=np.ascontiguousarray(lw1.astype(BF)),
                lw2=np.ascontiguousarray(lw2.astype(BF)), lb2=lb2,
                lw3=np.ascontiguousarray(lw3.astype(BF)), lb3=lb3,
                lw4=np.ascontiguousarray(lw4.astype(BF)), lb4=lb4,
                lw5=np.ascontiguousarray(lw5.astype(BF)), lb5=lb5)


def _prep_x13(x):
    """x [B,3,32,32] -> per-core [16, 100, 2*41*32] im2col-packed bf16."""
    f = np.float32
    xpad = np.zeros((B, 3, 44, 42), f)
    xpad[:, :, 5:37, 5:37] = x
    X = np.zeros((100, B, 41, 32), f)
    for dyo in range(3):
        for dx in range(11):
            for c in range(3):
                X[dyo * 33 + dx * 3 + c] = xpad[:, c, dyo:dyo + 41, dx:dx + 32]
    X[99] = 1.0
    out = []
    for r in range(N_CORES):
        pc = X[:, r * BC:(r + 1) * BC]  # [100, 32, 41, 32]
        pc = pc.reshape(100, 16, 2 * 41 * 32).transpose(1, 0, 2)
        out.append(np.ascontiguousarray(pc.astype(BF)))
    return out


def _prep_fc(fw1, fb1, fw2, fb2, fw3, fb3):
    f = np.float32
    outs = []
    for r in range(N_CORES):
        sl = slice(512 * r, 512 * (r + 1))
        # fw1s [128, (yx, cc, m)]: fw1[512r+m, (cc*128+k)*16+yx]
        fw1s = fw1[sl].reshape(512, 2, 128, 16).transpose(2, 3, 1, 0).reshape(128, -1)
        fb1s = fb1[sl].reshape(4, 128).T
        # fw2s [128, (kc, m)]: fw2[512r+m, kc*128+k]
        fw2s = fw2[sl].reshape(512, 32, 128).transpose(2, 1, 0).reshape(128, -1)
        fb2s = fb2[sl].reshape(4, 128).T
        # fw3s [128, (kc, m)]: fw3[m, 512r + kc*128 + k]
        fw3s = fw3[:, sl].reshape(100, 4, 128).transpose(2, 1, 0).reshape(128, -1)
        fb3s = (fb3 / N_CORES).reshape(100, 1)
        outs.append(dict(
            fw1s=np.ascontiguousarray(fw1s.astype(BF)),
            fb1s=np.ascontiguousarray(fb1s.astype(f)),
            fw2s=np.ascontiguousarray(fw2s.astype(BF)),
            fb2s=np.ascontiguousarray(fb2s.astype(f)),
            fw3s=np.ascontiguousarray(fw3s.astype(BF)),
            fb3s=np.ascontiguousarray(fb3s.astype(f)),
        ))
    return outs


_CACHE = {}

_SHAPES = dict(
    x13=(16, 100, 2 * 41 * 32), lw1=(100, 4 * 64),
    lw2=(128, 7 * 4 * 192), lb2=(128, 3),
    lw3=(128, 15360), lb3=(128, 3),
    lw4=(128, 27 * 256), lb4=(128, 2),
    lw5=(128, 18 * 256), lb5=(128, 2),
    fw1s=(128, 32 * 512), fb1s=(128, 4),
    fw2s=(128, 32 * 512), fb2s=(128, 4),
    fw3s=(128, 4 * 100), fb3s=(100, 1),
)

_BF16_INPUTS = {"x13", "lw1", "lw2", "lw3", "lw4", "lw5", "fw1s", "fw2s", "fw3s"}


def _build():
    if "nc" in _CACHE:
        return _CACHE["nc"]
    nc = bacc.Bacc("TRN2", target_bir_lowering=False, debug=False,
                   num_devices=N_CORES)

    def _dt(name):
        return BF16 if name in _BF16_INPUTS else F32
    t = {name: nc.dram_tensor(
            name, list(shape), _dt(name), kind="ExternalInput").ap()
         for name, shape in _SHAPES.items()}
    if DEBUG_DUMPS:
        for name, shape in _DBG_SHAPES.items():
            t[name] = nc.dram_tensor(
                name, list(shape), BF16, kind="ExternalOutput").ap()
    yout = nc.dram_tensor("yout", [100, B], BF16, kind="ExternalOutput").ap()
    with tile.TileContext(nc) as tc:
        _emit(nc, tc, t, yout)
    nc.compile()
    _CACHE["nc"] = nc
    return nc


def kernel(x, w1, b1, w2, b2, w3, b3, w4, b4, w5, b5,
           fw1, fb1, fw2, fb2, fw3, fb3):
    args = [np.asarray(a, np.float32) for a in
            (x, w1, b1, w2, b2, w3, b3, w4, b4, w5, b5, fw1, fb1, fw2, fb2, fw3, fb3)]
    (x, w1, b1, w2, b2, w3, b3, w4, b4, w5, b5,
     fw1, fb1, fw2, fb2, fw3, fb3) = args
    nc = _build()
    shared = _prep_shared(w1, b1, w2, b2, w3, b3, w4, b4, w5, b5)
    x13s = _prep_x13(x)
    fcs = _prep_fc(fw1, fb1, fw2, fb2, fw3, fb3)
    in_maps = [{**shared, "x13": x13s[r], **fcs[r]} for r in range(N_CORES)]
    res = run_bass_kernel_spmd(nc, in_maps, list(range(N_CORES)))
    _CACHE["last_results"] = res.results
    y = res.results[0]["yout"]  # [100, 256] bf16
    return np.ascontiguousarray(y.T.astype(np.float32))
